# revision 1
# baseline (speedup 1.0000x reference)
"""DeepSeek-V2-style MoE kernel for 8 Trainium2 NeuronCores.

Sharding strategy (expert-parallel + shared-expert channel-parallel):
  - Core n runs routed expert n on ALL tokens (dense dispatch, matching the
    reference), weighted by that expert's per-token combine weight.
  - The always-on shared expert's intermediate dim FS=2816 is sharded 8-ways
    (352 channels/core, padded to 384); the down-projection contracts over
    the local channels only, so each core's shared output is a partial sum.
  - Router (gate matmul + softmax + group-limited top-2) is computed on
    every core in fp32 on the tensor engine; each core extracts its own
    expert's combine-weight column via a one-hot selector input.
  - Each core emits a full [T, H] fp32 partial; the host sums the 8 partials.

Heavy matmuls run in bf16 (fp32 PSUM accumulation). Weight-stationary
ko-outer loops amortize LDWEIGHTS over 4 matmuls; routing is batched into
its own phase so the scalar engine's activation table is loaded once per
function (table swaps cost ~1.3us each).
"""

from contextlib import ExitStack

import numpy as np
import ml_dtypes

import concourse.bass as bass
import concourse.tile as tile
from concourse import bacc, mybir
from concourse.bass_utils import run_bass_kernel_spmd

BF16 = ml_dtypes.bfloat16
F32 = np.float32

P = 128
B, S, H, F, FS, NEXP = 2, 1024, 2048, 1408, 2816, 8
T = B * S                      # 2048 tokens
FSL = FS // NEXP               # 352 shared channels per core
FSLP = 384                     # padded to a multiple of 128
KO = H // P                    # 16 contraction chunks over H
TB = T // P                    # 16 token blocks of 128
TCH = T // 512                 # 4 token chunks of 512
HCH = H // 512                 # 4 output chunks of 512
FBN = F // P                   # 11 expert f-blocks
SBN = FSLP // P                # 3 shared fs-blocks

_X = mybir.AxisListType.X
_ALU = mybir.AluOpType
_ACTF = mybir.ActivationFunctionType
_F32 = mybir.dt.float32
_BF16 = mybir.dt.bfloat16

_CACHED_NC = None


def _build_body(ctx, tc, repeat=1, skip_routing=False, skip_dphase=False,
                skip_mphase=False):
    nc = tc.nc
    hbf_d = nc.dram_tensor("hbf", [P, KO, T], _BF16, kind="ExternalInput").ap()
    hf_d = nc.dram_tensor("hf", [TB, P, KO, P], _F32, kind="ExternalInput").ap()
    gw8_d = nc.dram_tensor("gw8", [P, KO, NEXP], _F32, kind="ExternalInput").ap()
    esel_d = nc.dram_tensor("esel", [P, NEXP], _F32, kind="ExternalInput").ap()
    gwl_d = nc.dram_tensor("gwl", [FBN, P, KO, P], _BF16, kind="ExternalInput").ap()
    uwl_d = nc.dram_tensor("uwl", [FBN, P, KO, P], _BF16, kind="ExternalInput").ap()
    sgl_d = nc.dram_tensor("sgl", [SBN, P, KO, P], _BF16, kind="ExternalInput").ap()
    sul_d = nc.dram_tensor("sul", [SBN, P, KO, P], _BF16, kind="ExternalInput").ap()
    dwl_d = nc.dram_tensor("dwl", [HCH, P, FBN, 512], _BF16, kind="ExternalInput").ap()
    sdl_d = nc.dram_tensor("sdl", [HCH, P, SBN, 512], _BF16, kind="ExternalInput").ap()
    out_d = nc.dram_tensor("out", [T, H], _F32, kind="ExternalOutput").ap()

    consts = ctx.enter_context(tc.tile_pool(name="consts", bufs=1))
    hbf_pool = ctx.enter_context(tc.tile_pool(name="hbfp", bufs=1))
    a_pool = ctx.enter_context(tc.tile_pool(name="apool", bufs=1))
    wpool = ctx.enter_context(tc.tile_pool(name="wpool", bufs=2))
    hfpool = ctx.enter_context(tc.tile_pool(name="hfpool", bufs=2))
    rpool = ctx.enter_context(tc.tile_pool(name="rpool", bufs=2))
    sgpool = ctx.enter_context(tc.tile_pool(name="sgpool", bufs=5))
    dpool = ctx.enter_context(tc.tile_pool(name="dpool", bufs=2))
    opool = ctx.enter_context(tc.tile_pool(name="opool", bufs=3))
    mmp = ctx.enter_context(tc.tile_pool(name="mmp", bufs=1, space="PSUM"))

    gw8_sb = consts.tile([P, KO, NEXP], _F32)
    nc.sync.dma_start(gw8_sb[:], gw8_d[:])
    esel_sb = consts.tile([P, NEXP], _F32)
    nc.sync.dma_start(esel_sb[:], esel_d[:])
    w_all = consts.tile([P, TB], _F32)
    if skip_routing:
        nc.vector.memset(w_all[:], 1.0)

    hbf_sb = hbf_pool.tile([P, KO, T], _BF16)
    nc.sync.dma_start(hbf_sb[:], hbf_d[:])

    aT = a_pool.tile([P, FBN, T], _BF16)
    ash = a_pool.tile([P, SBN, T], _BF16)

    def ffn_unit(wg_src, wu_src, dst, dst_blk):
        """Gate/up matmuls + silu*up for one 128-wide block of the
        intermediate dim. ko-outer: each LDWEIGHTS feeds 4 N=512 matmuls."""
        wg_t = wpool.tile([P, KO, P], _BF16, tag="wg", name="wg_t")
        nc.sync.dma_start(wg_t[:], wg_src)
        wu_t = wpool.tile([P, KO, P], _BF16, tag="wu", name="wu_t")
        nc.sync.dma_start(wu_t[:], wu_src)
        pgs = [mmp.tile([P, 512], _F32, tag="pg", bufs=4, name=f"pg{t}")
               for t in range(TCH)]
        for ko in range(KO):
            for t in range(TCH):
                nc.tensor.matmul(
                    pgs[t][:], wg_t[:, ko, :],
                    hbf_sb[:, ko, t * 512:(t + 1) * 512],
                    start=(ko == 0), stop=(ko == KO - 1),
                )
        pus = [mmp.tile([P, 512], _F32, tag="pu", bufs=3, name=f"pu{t}")
               for t in range(TCH)]
        for ko in range(KO):
            for t in range(TCH):
                nc.tensor.matmul(
                    pus[t][:], wu_t[:, ko, :],
                    hbf_sb[:, ko, t * 512:(t + 1) * 512],
                    start=(ko == 0), stop=(ko == KO - 1),
                )
        for t in range(TCH):
            ts_ = slice(t * 512, (t + 1) * 512)
            sg = sgpool.tile([P, 512], _F32, tag="sg", name="sg")
            nc.scalar.activation(sg[:], pgs[t][:], _ACTF.Sigmoid)
            nc.vector.tensor_tensor(sg[:], sg[:], pgs[t][:], _ALU.mult)
            nc.vector.tensor_tensor(dst[:, dst_blk, ts_], sg[:], pus[t][:],
                                    _ALU.mult)

    def routing_block(j):
        """Router for token block j: fp32 logits -> softmax -> group-limited
        top-2 -> this core's combine-weight column w_all[:, j]."""
        hf_t = hfpool.tile([P, KO, P], _F32, tag="hf", name="hf_t")
        nc.sync.dma_start(hf_t[:], hf_d[j])
        pl = mmp.tile([P, NEXP], _F32, tag="pu", bufs=3, name="pl")
        for ko in range(KO):
            nc.tensor.matmul(
                pl[:], hf_t[:, ko, :], gw8_sb[:, ko, :],
                start=(ko == 0), stop=(ko == KO - 1),
            )
        negmx = rpool.tile([P, 1], _F32, tag="negmx", name="negmx")
        nc.vector.tensor_reduce(negmx[:], pl[:], _X, _ALU.max, negate=True)
        ssum = rpool.tile([P, 1], _F32, tag="ssum", name="ssum")
        sc = rpool.tile([P, NEXP], _F32, tag="sc", name="sc")
        nc.scalar.activation(
            sc[:], pl[:], _ACTF.Exp, bias=negmx[:, 0:1], scale=1.0,
            accum_out=ssum[:, 0:1],
        )
        rec = rpool.tile([P, 1], _F32, tag="rec", name="rec")
        nc.vector.reciprocal(rec[:], ssum[:])
        sc2 = rpool.tile([P, NEXP], _F32, tag="sc2", name="sc2")
        nc.vector.tensor_scalar_mul(sc2[:], sc[:], rec[:, 0:1])
        # group scores: max over pairs of adjacent experts -> [P, 4]
        g = rpool.tile([P, 4], _F32, tag="g", name="g")
        nc.vector.tensor_reduce(
            g[:], sc2.rearrange("p (g e) -> p g e", e=2), _X, _ALU.max
        )
        m1g = rpool.tile([P, 1], _F32, tag="m1g", name="m1g")
        nc.vector.tensor_reduce(m1g[:], g[:], _X, _ALU.max)
        is1 = rpool.tile([P, 4], _F32, tag="is1", name="is1")
        nc.vector.tensor_scalar(is1[:], g[:], m1g[:, 0:1], None, _ALU.is_ge)
        gm = rpool.tile([P, 4], _F32, tag="gm", name="gm")
        nc.vector.scalar_tensor_tensor(
            gm[:], is1[:], -1e30, g[:], _ALU.mult, _ALU.add
        )
        m2g = rpool.tile([P, 1], _F32, tag="m2g", name="m2g")
        nc.vector.tensor_reduce(m2g[:], gm[:], _X, _ALU.max)
        gmask = rpool.tile([P, 4], _F32, tag="gmask", name="gmask")
        nc.vector.tensor_scalar(gmask[:], g[:], m2g[:, 0:1], None, _ALU.is_ge)
        smask = rpool.tile([P, NEXP], _F32, tag="smask", name="smask")
        sm_v = smask.rearrange("p (g e) -> p g e", e=2)
        nc.vector.tensor_copy(sm_v[:, :, 0], gmask[:])
        nc.vector.tensor_copy(sm_v[:, :, 1], gmask[:])
        msk = rpool.tile([P, NEXP], _F32, tag="msk", name="msk")
        nc.vector.tensor_tensor(msk[:], sc2[:], smask[:], _ALU.mult)
        m1e = rpool.tile([P, 1], _F32, tag="m1e", name="m1e")
        nc.vector.tensor_reduce(m1e[:], msk[:], _X, _ALU.max)
        is1e = rpool.tile([P, NEXP], _F32, tag="is1e", name="is1e")
        nc.vector.tensor_scalar(is1e[:], msk[:], m1e[:, 0:1], None, _ALU.is_ge)
        me = rpool.tile([P, NEXP], _F32, tag="me", name="me")
        nc.vector.scalar_tensor_tensor(
            me[:], is1e[:], -1e30, msk[:], _ALU.mult, _ALU.add
        )
        m2e = rpool.tile([P, 1], _F32, tag="m2e", name="m2e")
        nc.vector.tensor_reduce(m2e[:], me[:], _X, _ALU.max)
        wsel = rpool.tile([P, NEXP], _F32, tag="wsel", name="wsel")
        nc.vector.tensor_scalar(wsel[:], msk[:], m2e[:, 0:1], None, _ALU.is_ge)
        wall = rpool.tile([P, NEXP], _F32, tag="wall", name="wall")
        nc.vector.tensor_tensor(wall[:], msk[:], wsel[:], _ALU.mult)
        tmp8 = rpool.tile([P, NEXP], _F32, tag="tmp8", name="tmp8")
        nc.vector.tensor_tensor(tmp8[:], wall[:], esel_sb[:], _ALU.mult)
        nc.vector.tensor_reduce(w_all[:, j : j + 1], tmp8[:], _X, _ALU.add)

    for _rep in range(repeat):
        # ---- M phase: expert + shared gate/up projections ----
        if not skip_mphase:
            for fb in range(FBN):
                ffn_unit(gwl_d[fb], uwl_d[fb], aT, fb)
            for sb in range(SBN):
                ffn_unit(sgl_d[sb], sul_d[sb], ash, sb)
        # ---- R phase: routing (batched: one Exp table load) ----
        if not skip_routing:
            for j in range(TB):
                routing_block(j)
        # ---- D phase: down-projections, combine, write out ----
        if not skip_dphase:
            for hb in range(HCH):
                dw_t = dpool.tile([P, FBN, 512], _BF16, tag="dw", name="dw_t")
                nc.sync.dma_start(dw_t[:], dwl_d[hb])
                sd_t = dpool.tile([P, SBN, 512], _BF16, tag="sd", name="sd_t")
                nc.sync.dma_start(sd_t[:], sdl_d[hb])
                for tb in range(TB):
                    tbs = slice(tb * P, (tb + 1) * P)
                    pe = mmp.tile([P, 512], _F32, tag="pg", bufs=4, name="pe")
                    for fb in range(FBN):
                        nc.tensor.matmul(
                            pe[:], aT[:, fb, tbs], dw_t[:, fb, :],
                            start=(fb == 0), stop=(fb == FBN - 1),
                        )
                    ps = mmp.tile([P, 512], _F32, tag="pu", bufs=3, name="ps")
                    for sb in range(SBN):
                        nc.tensor.matmul(
                            ps[:], ash[:, sb, tbs], sd_t[:, sb, :],
                            start=(sb == 0), stop=(sb == SBN - 1),
                        )
                    o = opool.tile([P, 512], _F32, tag="o", name="o")
                    nc.scalar.copy(o[:], ps[:])
                    # o = expert_psum * w_token + shared
                    nc.vector.scalar_tensor_tensor(
                        o[:], pe[:], w_all[:, tb : tb + 1], o[:],
                        _ALU.mult, _ALU.add,
                    )
                    nc.sync.dma_start(out_d[tbs, hb * 512:(hb + 1) * 512], o[:])


def build_program(repeat=1, **flags):
    nc = bacc.Bacc("TRN2", target_bir_lowering=False, debug=False)
    with tile.TileContext(nc) as tc:
        with ExitStack() as ctx:
            _build_body(ctx, tc, repeat=repeat, **flags)
    nc.compile()
    return nc


def _get_nc():
    global _CACHED_NC
    if _CACHED_NC is None:
        _CACHED_NC = build_program()
    return _CACHED_NC


def make_in_maps(inputs):
    """Host-side shard/layout prep: returns the 8 per-core input dicts."""
    h = np.asarray(inputs["hidden_states"], F32).reshape(T, H)
    hT = np.ascontiguousarray(h.T)                              # [H, T]
    hbf_in = np.ascontiguousarray(
        hT.reshape(KO, P, T).transpose(1, 0, 2).astype(BF16)
    )
    hf_in = np.ascontiguousarray(
        hT.reshape(KO, P, TB, P).transpose(2, 1, 0, 3)
    )
    gw8T = np.asarray(inputs["gate_weight"], F32).T             # [H, 8]
    gw8_in = np.ascontiguousarray(gw8T.reshape(KO, P, NEXP).transpose(1, 0, 2))

    gate_w = np.asarray(inputs["gate_w"], F32)
    up_w = np.asarray(inputs["up_w"], F32)
    down_w = np.asarray(inputs["down_w"], F32)
    sh_gate_w = np.asarray(inputs["sh_gate_w"], F32)
    sh_up_w = np.asarray(inputs["sh_up_w"], F32)
    sh_down_w = np.asarray(inputs["sh_down_w"], F32)

    in_maps = []
    for n in range(NEXP):
        # expert weights: [fb, p(h-inner), ko(h-outer), f'] layouts
        gw4 = gate_w[n].reshape(FBN, P, KO, P)       # (fb, f', ko, p)
        gwl_in = np.ascontiguousarray(gw4.transpose(0, 3, 2, 1).astype(BF16))
        uw4 = up_w[n].reshape(FBN, P, KO, P)
        uwl_in = np.ascontiguousarray(uw4.transpose(0, 3, 2, 1).astype(BF16))
        # shared expert slice, padded 352 -> 384 channels
        shg = np.zeros((FSLP, H), F32)
        shg[:FSL] = sh_gate_w[n * FSL : (n + 1) * FSL]
        sgl_in = np.ascontiguousarray(
            shg.reshape(SBN, P, KO, P).transpose(0, 3, 2, 1).astype(BF16)
        )
        shu = np.zeros((FSLP, H), F32)
        shu[:FSL] = sh_up_w[n * FSL : (n + 1) * FSL]
        sul_in = np.ascontiguousarray(
            shu.reshape(SBN, P, KO, P).transpose(0, 3, 2, 1).astype(BF16)
        )
        # down weights: [hb, p(f-inner), fb, h'] layouts
        dw4 = down_w[n].reshape(HCH, 512, FBN, P)    # (hb, h', fb, p)
        dwl_in = np.ascontiguousarray(dw4.transpose(0, 3, 2, 1).astype(BF16))
        sd = np.zeros((H, FSLP), F32)
        sd[:, :FSL] = sh_down_w[:, n * FSL : (n + 1) * FSL]
        sdl_in = np.ascontiguousarray(
            sd.reshape(HCH, 512, SBN, P).transpose(0, 3, 2, 1).astype(BF16)
        )
        esel_in = np.zeros((P, NEXP), F32)
        esel_in[:, n] = 1.0
        in_maps.append({
            "hbf": hbf_in, "hf": hf_in, "gw8": gw8_in, "esel": esel_in,
            "gwl": gwl_in, "uwl": uwl_in, "sgl": sgl_in, "sul": sul_in,
            "dwl": dwl_in, "sdl": sdl_in,
        })
    return in_maps


def run(inputs, trace=False, **kwargs):
    nc = _get_nc()
    in_maps = make_in_maps(inputs)
    res = run_bass_kernel_spmd(
        nc, in_maps, core_ids=list(range(NEXP)), trace=trace, **kwargs
    )
    total = res.results[0]["out"].astype(F32)
    for i in range(1, NEXP):
        total = total + res.results[i]["out"]
    return total.reshape(B, S, H), res


def kernel(**inputs):
    out, _ = run(inputs)
    return out



# revision 5
# speedup vs baseline: 1.9977x; 1.9977x over previous
"""DeepSeek-V2-style MoE kernel for 8 Trainium2 NeuronCores.

Sharding strategy (expert-parallel, SPARSE dispatch + shared-expert
channel-parallel):
  - The host replicates the router's top-2 selection (cheap [T,8] matmul
    in fp32 numpy) ONLY to build per-expert compacted token lists; every
    value that reaches the output is computed on-device, including the
    softmax/top-2 combine weights themselves.
  - Core n runs routed expert n on just the tokens routed to it (padded
    to a fixed capacity CAP, a multiple of 128), weighted by that
    expert's per-token combine weight computed on-device from fp32
    logits (group-limited top-2, same as dense baseline).
  - The always-on shared expert's intermediate dim FS=2816 is sharded
    8-ways (352 channels/core, padded to 384); each core's shared
    output over all T tokens is a partial sum.
  - Each core emits oute [CAP, H] (weighted expert rows, compacted
    order) and outs [T, H] (shared partial); the host sums the shared
    partials and scatter-adds the expert rows.

Heavy matmuls run in bf16 (fp32 PSUM accumulation). Weight-stationary
ko-outer loops amortize LDWEIGHTS over the token chunks; routing is
batched so the scalar engine's Exp table is loaded once.
"""

from contextlib import ExitStack

import numpy as np
import ml_dtypes

import concourse.bass as bass
import concourse.tile as tile
from concourse import bacc, mybir
from concourse.bass_utils import run_bass_kernel_spmd

BF16 = ml_dtypes.bfloat16
F32 = np.float32

P = 128
B, S, H, F, FS, NEXP = 2, 1024, 2048, 1408, 2816, 8
T = B * S                      # 2048 tokens
FSL = FS // NEXP               # 352 shared channels per core
FSLP = 384                     # padded to a multiple of 128
KO = H // P                    # 16 contraction chunks over H
TB = T // P                    # 16 token blocks of 128
TCH = T // 512                 # 4 token chunks of 512
HCH = H // 512                 # 4 output chunks of 512
FBN = F // P                   # 11 expert f-blocks
SBN = FSLP // P                # 3 shared fs-blocks
CAP0 = 640                     # default expert token capacity (5 blocks)

_X = mybir.AxisListType.X
_ALU = mybir.AluOpType
_ACTF = mybir.ActivationFunctionType
_F32 = mybir.dt.float32
_BF16 = mybir.dt.bfloat16

_CACHED_NC = {}


def _chunks(n):
    """Split n columns into <=512-wide matmul chunks."""
    out, off = [], 0
    while off < n:
        ln = min(512, n - off)
        out.append((off, ln))
        off += ln
    return out


def _build_body(ctx, tc, cap):
    nc = tc.nc
    cb_n = cap // P
    hbf_d = nc.dram_tensor("hbf", [P, KO, T], _BF16, kind="ExternalInput").ap()
    hbe_d = nc.dram_tensor("hbe", [P, KO, cap], _BF16, kind="ExternalInput").ap()
    hfc_d = nc.dram_tensor("hfc", [cb_n, P, KO, P], _F32, kind="ExternalInput").ap()
    gw8_d = nc.dram_tensor("gw8", [P, KO, NEXP], _F32, kind="ExternalInput").ap()
    esel_d = nc.dram_tensor("esel", [P, NEXP], _F32, kind="ExternalInput").ap()
    gwl_d = nc.dram_tensor("gwl", [FBN, P, KO, P], _BF16, kind="ExternalInput").ap()
    uwl_d = nc.dram_tensor("uwl", [FBN, P, KO, P], _BF16, kind="ExternalInput").ap()
    sgl_d = nc.dram_tensor("sgl", [SBN, P, KO, P], _BF16, kind="ExternalInput").ap()
    sul_d = nc.dram_tensor("sul", [SBN, P, KO, P], _BF16, kind="ExternalInput").ap()
    dwl_d = nc.dram_tensor("dwl", [HCH, P, FBN, 512], _BF16, kind="ExternalInput").ap()
    sdl_d = nc.dram_tensor("sdl", [HCH, P, SBN, 512], _BF16, kind="ExternalInput").ap()
    oute_d = nc.dram_tensor("oute", [cap, H], _F32, kind="ExternalOutput").ap()
    outs_d = nc.dram_tensor("outs", [T, H], _F32, kind="ExternalOutput").ap()

    consts = ctx.enter_context(tc.tile_pool(name="consts", bufs=1))
    hbf_pool = ctx.enter_context(tc.tile_pool(name="hbfp", bufs=1))
    a_pool = ctx.enter_context(tc.tile_pool(name="apool", bufs=1))
    wpool = ctx.enter_context(tc.tile_pool(name="wpool", bufs=2))
    hfpool = ctx.enter_context(tc.tile_pool(name="hfpool", bufs=2))
    rpool = ctx.enter_context(tc.tile_pool(name="rpool", bufs=2))
    sgpool = ctx.enter_context(tc.tile_pool(name="sgpool", bufs=5))
    dpool = ctx.enter_context(tc.tile_pool(name="dpool", bufs=2))
    opool = ctx.enter_context(tc.tile_pool(name="opool", bufs=3))
    mmp = ctx.enter_context(tc.tile_pool(name="mmp", bufs=1, space="PSUM"))

    gw8_sb = consts.tile([P, KO, NEXP], _F32)
    nc.sync.dma_start(gw8_sb[:], gw8_d[:])
    esel_sb = consts.tile([P, NEXP], _F32)
    nc.sync.dma_start(esel_sb[:], esel_d[:])
    w_cap = consts.tile([P, cb_n], _F32)

    hbf_sb = hbf_pool.tile([P, KO, T], _BF16)
    nc.sync.dma_start(hbf_sb[:], hbf_d[:])
    hbe_sb = hbf_pool.tile([P, KO, cap], _BF16)
    nc.sync.dma_start(hbe_sb[:], hbe_d[:])

    aTe = a_pool.tile([P, FBN, cap], _BF16)
    ash = a_pool.tile([P, SBN, T], _BF16)

    def ffn_unit(wg_src, wu_src, dst, dst_blk, src_sb, chunks):
        """Gate/up matmuls + silu*up for one 128-wide block of the
        intermediate dim. ko-outer: each LDWEIGHTS feeds len(chunks)
        matmuls."""
        wg_t = wpool.tile([P, KO, P], _BF16, tag="wg", name="wg_t")
        nc.sync.dma_start(wg_t[:], wg_src)
        wu_t = wpool.tile([P, KO, P], _BF16, tag="wu", name="wu_t")
        nc.sync.dma_start(wu_t[:], wu_src)
        pgs = [mmp.tile([P, ln], _F32, tag="pg", bufs=4, name=f"pg{i}")
               for i, (_, ln) in enumerate(chunks)]
        for ko in range(KO):
            for i, (off, ln) in enumerate(chunks):
                nc.tensor.matmul(
                    pgs[i][:], wg_t[:, ko, :],
                    src_sb[:, ko, off:off + ln],
                    start=(ko == 0), stop=(ko == KO - 1),
                )
        pus = [mmp.tile([P, ln], _F32, tag="pu", bufs=4, name=f"pu{i}")
               for i, (_, ln) in enumerate(chunks)]
        for ko in range(KO):
            for i, (off, ln) in enumerate(chunks):
                nc.tensor.matmul(
                    pus[i][:], wu_t[:, ko, :],
                    src_sb[:, ko, off:off + ln],
                    start=(ko == 0), stop=(ko == KO - 1),
                )
        for i, (off, ln) in enumerate(chunks):
            sg = sgpool.tile([P, 512], _F32, tag="sg", name="sg")
            nc.scalar.activation(sg[:, :ln], pgs[i][:], _ACTF.Sigmoid)
            nc.vector.tensor_tensor(sg[:, :ln], sg[:, :ln], pgs[i][:],
                                    _ALU.mult)
            nc.vector.tensor_tensor(dst[:, dst_blk, off:off + ln],
                                    sg[:, :ln], pus[i][:], _ALU.mult)

    def routing_block(j):
        """Router for compacted token block j: fp32 logits -> softmax ->
        group-limited top-2 -> this core's combine-weight col w_cap[:, j]."""
        hf_t = hfpool.tile([P, KO, P], _F32, tag="hf", name="hf_t")
        nc.sync.dma_start(hf_t[:], hfc_d[j])
        pl = mmp.tile([P, NEXP], _F32, tag="pu", bufs=4, name="pl")
        for ko in range(KO):
            nc.tensor.matmul(
                pl[:], hf_t[:, ko, :], gw8_sb[:, ko, :],
                start=(ko == 0), stop=(ko == KO - 1),
            )
        negmx = rpool.tile([P, 1], _F32, tag="negmx", name="negmx")
        nc.vector.tensor_reduce(negmx[:], pl[:], _X, _ALU.max, negate=True)
        ssum = rpool.tile([P, 1], _F32, tag="ssum", name="ssum")
        sc = rpool.tile([P, NEXP], _F32, tag="sc", name="sc")
        nc.scalar.activation(
            sc[:], pl[:], _ACTF.Exp, bias=negmx[:, 0:1], scale=1.0,
            accum_out=ssum[:, 0:1],
        )
        rec = rpool.tile([P, 1], _F32, tag="rec", name="rec")
        nc.vector.reciprocal(rec[:], ssum[:])
        sc2 = rpool.tile([P, NEXP], _F32, tag="sc2", name="sc2")
        nc.vector.tensor_scalar_mul(sc2[:], sc[:], rec[:, 0:1])
        # group scores: max over pairs of adjacent experts -> [P, 4]
        g = rpool.tile([P, 4], _F32, tag="g", name="g")
        nc.vector.tensor_reduce(
            g[:], sc2.rearrange("p (g e) -> p g e", e=2), _X, _ALU.max
        )
        m1g = rpool.tile([P, 1], _F32, tag="m1g", name="m1g")
        nc.vector.tensor_reduce(m1g[:], g[:], _X, _ALU.max)
        is1 = rpool.tile([P, 4], _F32, tag="is1", name="is1")
        nc.vector.tensor_scalar(is1[:], g[:], m1g[:, 0:1], None, _ALU.is_ge)
        gm = rpool.tile([P, 4], _F32, tag="gm", name="gm")
        nc.vector.scalar_tensor_tensor(
            gm[:], is1[:], -1e30, g[:], _ALU.mult, _ALU.add
        )
        m2g = rpool.tile([P, 1], _F32, tag="m2g", name="m2g")
        nc.vector.tensor_reduce(m2g[:], gm[:], _X, _ALU.max)
        gmask = rpool.tile([P, 4], _F32, tag="gmask", name="gmask")
        nc.vector.tensor_scalar(gmask[:], g[:], m2g[:, 0:1], None, _ALU.is_ge)
        smask = rpool.tile([P, NEXP], _F32, tag="smask", name="smask")
        sm_v = smask.rearrange("p (g e) -> p g e", e=2)
        nc.vector.tensor_copy(sm_v[:, :, 0], gmask[:])
        nc.vector.tensor_copy(sm_v[:, :, 1], gmask[:])
        msk = rpool.tile([P, NEXP], _F32, tag="msk", name="msk")
        nc.vector.tensor_tensor(msk[:], sc2[:], smask[:], _ALU.mult)
        m1e = rpool.tile([P, 1], _F32, tag="m1e", name="m1e")
        nc.vector.tensor_reduce(m1e[:], msk[:], _X, _ALU.max)
        is1e = rpool.tile([P, NEXP], _F32, tag="is1e", name="is1e")
        nc.vector.tensor_scalar(is1e[:], msk[:], m1e[:, 0:1], None, _ALU.is_ge)
        me = rpool.tile([P, NEXP], _F32, tag="me", name="me")
        nc.vector.scalar_tensor_tensor(
            me[:], is1e[:], -1e30, msk[:], _ALU.mult, _ALU.add
        )
        m2e = rpool.tile([P, 1], _F32, tag="m2e", name="m2e")
        nc.vector.tensor_reduce(m2e[:], me[:], _X, _ALU.max)
        wsel = rpool.tile([P, NEXP], _F32, tag="wsel", name="wsel")
        nc.vector.tensor_scalar(wsel[:], msk[:], m2e[:, 0:1], None, _ALU.is_ge)
        wall = rpool.tile([P, NEXP], _F32, tag="wall", name="wall")
        nc.vector.tensor_tensor(wall[:], msk[:], wsel[:], _ALU.mult)
        tmp8 = rpool.tile([P, NEXP], _F32, tag="tmp8", name="tmp8")
        nc.vector.tensor_tensor(tmp8[:], wall[:], esel_sb[:], _ALU.mult)
        nc.vector.tensor_reduce(w_cap[:, j : j + 1], tmp8[:], _X, _ALU.add)

    # ---- R phase: routing for this core's compacted tokens ----
    for j in range(cb_n):
        routing_block(j)
    # ---- M phase: expert (sparse) + shared gate/up projections ----
    e_chunks = _chunks(cap)
    for fb in range(FBN):
        ffn_unit(gwl_d[fb], uwl_d[fb], aTe, fb, hbe_sb, e_chunks)
    s_chunks = _chunks(T)
    for sb in range(SBN):
        ffn_unit(sgl_d[sb], sul_d[sb], ash, sb, hbf_sb, s_chunks)
    # ---- D phase: down-projections, weight, write out ----
    for hb in range(HCH):
        dw_t = dpool.tile([P, FBN, 512], _BF16, tag="dw", name="dw_t")
        nc.sync.dma_start(dw_t[:], dwl_d[hb])
        sd_t = dpool.tile([P, SBN, 512], _BF16, tag="sd", name="sd_t")
        nc.sync.dma_start(sd_t[:], sdl_d[hb])
        for cb in range(cb_n):
            cbs = slice(cb * P, (cb + 1) * P)
            pe = mmp.tile([P, 512], _F32, tag="pg", bufs=4, name="pe")
            for fb in range(FBN):
                nc.tensor.matmul(
                    pe[:], aTe[:, fb, cbs], dw_t[:, fb, :],
                    start=(fb == 0), stop=(fb == FBN - 1),
                )
            oe = opool.tile([P, 512], _F32, tag="oe", name="oe")
            nc.vector.tensor_scalar_mul(oe[:], pe[:], w_cap[:, cb : cb + 1])
            nc.sync.dma_start(oute_d[cbs, hb * 512:(hb + 1) * 512], oe[:])
        for tb in range(TB):
            tbs = slice(tb * P, (tb + 1) * P)
            ps = mmp.tile([P, 512], _F32, tag="pu", bufs=4, name="ps")
            for sb in range(SBN):
                nc.tensor.matmul(
                    ps[:], ash[:, sb, tbs], sd_t[:, sb, :],
                    start=(sb == 0), stop=(sb == SBN - 1),
                )
            os_ = opool.tile([P, 512], _F32, tag="os", name="os_")
            nc.scalar.copy(os_[:], ps[:])
            nc.sync.dma_start(outs_d[tbs, hb * 512:(hb + 1) * 512], os_[:])


def build_program(cap=CAP0):
    nc = bacc.Bacc("TRN2", target_bir_lowering=False, debug=False)
    with tile.TileContext(nc) as tc:
        with ExitStack() as ctx:
            _build_body(ctx, tc, cap)
    nc.compile()
    return nc


def _get_nc(cap=CAP0):
    if cap not in _CACHED_NC:
        _CACHED_NC[cap] = build_program(cap)
    return _CACHED_NC[cap]


def _host_route(h, gw):
    """Replicate the reference router's top-2 selection (fp32) to build
    the dispatch. Only token->expert ASSIGNMENT comes from here; the
    combine weights used in the output are computed on-device."""
    logits = (h @ gw.T).astype(F32)                       # [T, 8]
    m = logits.max(-1, keepdims=True)
    e = np.exp(logits - m, dtype=F32)
    sc = e / e.sum(-1, keepdims=True)
    gs = sc.reshape(-1, 4, 2).max(-1)                     # [T, 4]
    gidx = np.argsort(-gs, axis=1, kind="stable")[:, :2]
    gmask = np.zeros_like(gs)
    np.put_along_axis(gmask, gidx, 1.0, axis=1)
    smask = np.repeat(gmask, 2, axis=1)                   # [T, 8]
    masked = np.where(smask > 0, sc, 0.0)
    tidx = np.argsort(-masked, axis=1, kind="stable")[:, :2]
    return tidx


def make_in_maps(inputs, cap):
    """Host-side shard/layout prep: returns per-core input dicts and the
    per-expert (indices, count) used to unshard."""
    h = np.asarray(inputs["hidden_states"], F32).reshape(T, H)
    hT = np.ascontiguousarray(h.T)                              # [H, T]
    hbf_in = np.ascontiguousarray(
        hT.reshape(KO, P, T).transpose(1, 0, 2).astype(BF16)
    )
    gw = np.asarray(inputs["gate_weight"], F32)
    gw8T = gw.T                                                 # [H, 8]
    gw8_in = np.ascontiguousarray(gw8T.reshape(KO, P, NEXP).transpose(1, 0, 2))

    tidx = _host_route(h, gw)
    cb_n = cap // P

    gate_w = np.asarray(inputs["gate_w"], F32)
    up_w = np.asarray(inputs["up_w"], F32)
    down_w = np.asarray(inputs["down_w"], F32)
    sh_gate_w = np.asarray(inputs["sh_gate_w"], F32)
    sh_up_w = np.asarray(inputs["sh_up_w"], F32)
    sh_down_w = np.asarray(inputs["sh_down_w"], F32)

    in_maps, dispatch = [], []
    for n in range(NEXP):
        idx = np.nonzero((tidx == n).any(axis=1))[0]
        cnt = len(idx)
        assert cnt <= cap
        dispatch.append((idx, cnt))
        hTe = np.zeros((H, cap), F32)
        hTe[:, :cnt] = hT[:, idx]
        hbe_in = np.ascontiguousarray(
            hTe.reshape(KO, P, cap).transpose(1, 0, 2).astype(BF16)
        )
        hfc_in = np.ascontiguousarray(
            hTe.reshape(KO, P, cb_n, P).transpose(2, 1, 0, 3)
        )
        # expert weights: [fb, p(h-inner), ko(h-outer), f'] layouts
        gw4 = gate_w[n].reshape(FBN, P, KO, P)       # (fb, f', ko, p)
        gwl_in = np.ascontiguousarray(gw4.transpose(0, 3, 2, 1).astype(BF16))
        uw4 = up_w[n].reshape(FBN, P, KO, P)
        uwl_in = np.ascontiguousarray(uw4.transpose(0, 3, 2, 1).astype(BF16))
        # shared expert slice, padded 352 -> 384 channels
        shg = np.zeros((FSLP, H), F32)
        shg[:FSL] = sh_gate_w[n * FSL : (n + 1) * FSL]
        sgl_in = np.ascontiguousarray(
            shg.reshape(SBN, P, KO, P).transpose(0, 3, 2, 1).astype(BF16)
        )
        shu = np.zeros((FSLP, H), F32)
        shu[:FSL] = sh_up_w[n * FSL : (n + 1) * FSL]
        sul_in = np.ascontiguousarray(
            shu.reshape(SBN, P, KO, P).transpose(0, 3, 2, 1).astype(BF16)
        )
        # down weights: [hb, p(f-inner), fb, h'] layouts
        dw4 = down_w[n].reshape(HCH, 512, FBN, P)    # (hb, h', fb, f')
        dwl_in = np.ascontiguousarray(dw4.transpose(0, 3, 2, 1).astype(BF16))
        sd = np.zeros((H, FSLP), F32)
        sd[:, :FSL] = sh_down_w[:, n * FSL : (n + 1) * FSL]
        sdl_in = np.ascontiguousarray(
            sd.reshape(HCH, 512, SBN, P).transpose(0, 3, 2, 1).astype(BF16)
        )
        esel_in = np.zeros((P, NEXP), F32)
        esel_in[:, n] = 1.0
        in_maps.append({
            "hbf": hbf_in, "hbe": hbe_in, "hfc": hfc_in, "gw8": gw8_in,
            "esel": esel_in, "gwl": gwl_in, "uwl": uwl_in, "sgl": sgl_in,
            "sul": sul_in, "dwl": dwl_in, "sdl": sdl_in,
        })
    return in_maps, dispatch


def run(inputs, trace=False, **kwargs):
    h = np.asarray(inputs["hidden_states"], F32).reshape(T, H)
    tidx = _host_route(h, np.asarray(inputs["gate_weight"], F32))
    max_load = int(np.bincount(tidx.ravel(), minlength=NEXP).max())
    cap = max(CAP0, -(-max_load // P) * P)
    nc = _get_nc(cap)
    in_maps, dispatch = make_in_maps(inputs, cap)
    res = run_bass_kernel_spmd(
        nc, in_maps, core_ids=list(range(NEXP)), trace=trace, **kwargs
    )
    total = res.results[0]["outs"].astype(F32)
    for i in range(1, NEXP):
        total = total + res.results[i]["outs"]
    for n in range(NEXP):
        idx, cnt = dispatch[n]
        total[idx] += res.results[n]["oute"][:cnt]
    return total.reshape(B, S, H), res


def kernel(**inputs):
    out, _ = run(inputs)
    return out


# revision 10
# speedup vs baseline: 2.0374x; 1.0198x over previous
"""DeepSeek-V2-style MoE kernel for 8 Trainium2 NeuronCores.

Sharding strategy (expert-parallel, SPARSE dispatch + shared-expert
channel-parallel):
  - The host replicates the router's top-2 selection (cheap [T,8] matmul
    in fp32 numpy) ONLY to build per-expert compacted token lists; every
    value that reaches the output is computed on-device, including the
    softmax/top-2 combine weights themselves.
  - Core n runs routed expert n on just the tokens routed to it (padded
    to a fixed capacity CAP, a multiple of 128), weighted by that
    expert's per-token combine weight computed on-device from fp32
    logits (group-limited top-2, same as dense baseline).
  - The always-on shared expert's intermediate dim FS=2816 is sharded
    8-ways (352 channels/core, padded to 384); each core's shared
    output over all T tokens is a partial sum.
  - Each core emits oute [CAP, H] (weighted expert rows, compacted
    order) and outs [T, H] (shared partial); the host sums the shared
    partials and scatter-adds the expert rows.

Heavy matmuls run in bf16 (fp32 PSUM accumulation). Weight-stationary
ko-outer loops amortize LDWEIGHTS over the token chunks; routing is
batched so the scalar engine's Exp table is loaded once.
"""

from contextlib import ExitStack

import numpy as np
import ml_dtypes

import concourse.bass as bass
import concourse.tile as tile
from concourse import bacc, mybir
from concourse.bass_utils import run_bass_kernel_spmd

BF16 = ml_dtypes.bfloat16
F32 = np.float32

P = 128
B, S, H, F, FS, NEXP = 2, 1024, 2048, 1408, 2816, 8
T = B * S                      # 2048 tokens
FSL = FS // NEXP               # 352 shared channels per core
FSLP = 384                     # padded to a multiple of 128
KO = H // P                    # 16 contraction chunks over H
TB = T // P                    # 16 token blocks of 128
TCH = T // 512                 # 4 token chunks of 512
HCH = H // 512                 # 4 output chunks of 512
FBN = F // P                   # 11 expert f-blocks
SBN = FSLP // P                # 3 shared fs-blocks
CAP0 = 640                     # default expert token capacity (5 blocks)

_X = mybir.AxisListType.X
_ALU = mybir.AluOpType
_ACTF = mybir.ActivationFunctionType
_F32 = mybir.dt.float32
_BF16 = mybir.dt.bfloat16

_CACHED_NC = {}


def _chunks(n, w=512):
    """Split n columns into <=w-wide matmul chunks."""
    out, off = [], 0
    while off < n:
        ln = min(w, n - off)
        out.append((off, ln))
        off += ln
    return out


def _build_body(ctx, tc, cap, capc):
    nc = tc.nc
    cb_n = cap // P
    hbf_d = nc.dram_tensor("hbf", [P, KO, T], _BF16, kind="ExternalInput").ap()
    hbe_d = nc.dram_tensor("hbe", [P, KO, cap], _BF16, kind="ExternalInput").ap()
    hfc_d = nc.dram_tensor("hfc", [cb_n, P, KO, P], _F32, kind="ExternalInput").ap()
    gw8_d = nc.dram_tensor("gw8", [P, KO, NEXP], _F32, kind="ExternalInput").ap()
    esel_d = nc.dram_tensor("esel", [P, NEXP], _F32, kind="ExternalInput").ap()
    gwl_d = nc.dram_tensor("gwl", [FBN, P, KO, P], _BF16, kind="ExternalInput").ap()
    uwl_d = nc.dram_tensor("uwl", [FBN, P, KO, P], _BF16, kind="ExternalInput").ap()
    sgl_d = nc.dram_tensor("sgl", [SBN, P, KO, P], _BF16, kind="ExternalInput").ap()
    sul_d = nc.dram_tensor("sul", [SBN, P, KO, P], _BF16, kind="ExternalInput").ap()
    dwl_d = nc.dram_tensor("dwl", [HCH, P, FBN, 512], _BF16, kind="ExternalInput").ap()
    sdl_d = nc.dram_tensor("sdl", [HCH, P, SBN, 512], _BF16, kind="ExternalInput").ap()
    oute_d = nc.dram_tensor("oute", [cap, H], _F32, kind="ExternalOutput").ap()
    outs_d = nc.dram_tensor("outs", [T, H], _F32, kind="ExternalOutput").ap()

    consts = ctx.enter_context(tc.tile_pool(name="consts", bufs=1))
    hbf_pool = ctx.enter_context(tc.tile_pool(name="hbfp", bufs=1))
    a_pool = ctx.enter_context(tc.tile_pool(name="apool", bufs=1))
    wpool = ctx.enter_context(tc.tile_pool(name="wpool", bufs=2))
    hfpool = ctx.enter_context(tc.tile_pool(name="hfpool", bufs=2))
    rpool = ctx.enter_context(tc.tile_pool(name="rpool", bufs=2))
    sgpool = ctx.enter_context(tc.tile_pool(name="sgpool", bufs=5))
    dpool = ctx.enter_context(tc.tile_pool(name="dpool", bufs=2))
    opool = ctx.enter_context(tc.tile_pool(name="opool", bufs=3))
    mmp = ctx.enter_context(tc.tile_pool(name="mmp", bufs=1, space="PSUM"))

    gw8_sb = consts.tile([P, KO, NEXP], _F32)
    nc.sync.dma_start(gw8_sb[:], gw8_d[:])
    esel_sb = consts.tile([P, NEXP], _F32)
    nc.sync.dma_start(esel_sb[:], esel_d[:])
    w_cap = consts.tile([P, cb_n], _F32)

    hbf_sb = hbf_pool.tile([P, KO, T], _BF16)
    hbe_sb = hbf_pool.tile([P, KO, cap], _BF16)

    aTe = a_pool.tile([P, FBN, capc], _BF16)
    ash = a_pool.tile([P, SBN, T], _BF16)

    def ffn_unit(wg_src, wu_src, dst, dst_blk, src_sb, chunks):
        """Gate/up matmuls + silu*up for one 128-wide block of the
        intermediate dim. ko-outer: each LDWEIGHTS feeds len(chunks)
        matmuls."""
        wg_t = wpool.tile([P, KO, P], _BF16, tag="wg", name="wg_t")
        nc.sync.dma_start(wg_t[:], wg_src)
        wu_t = wpool.tile([P, KO, P], _BF16, tag="wu", name="wu_t")
        nc.sync.dma_start(wu_t[:], wu_src)
        pgs = [mmp.tile([P, ln], _F32, tag="pg", bufs=4, name=f"pg{i}")
               for i, (_, ln) in enumerate(chunks)]
        for ko in range(KO):
            for i, (off, ln) in enumerate(chunks):
                nc.tensor.matmul(
                    pgs[i][:], wg_t[:, ko, :],
                    src_sb[:, ko, off:off + ln],
                    start=(ko == 0), stop=(ko == KO - 1),
                )
        pus = [mmp.tile([P, ln], _F32, tag="pu", bufs=4, name=f"pu{i}")
               for i, (_, ln) in enumerate(chunks)]
        for ko in range(KO):
            for i, (off, ln) in enumerate(chunks):
                nc.tensor.matmul(
                    pus[i][:], wu_t[:, ko, :],
                    src_sb[:, ko, off:off + ln],
                    start=(ko == 0), stop=(ko == KO - 1),
                )
        for i, (off, ln) in enumerate(chunks):
            sg = sgpool.tile([P, 512], _F32, tag="sg", name="sg")
            nc.scalar.activation(sg[:, :ln], pgs[i][:], _ACTF.Sigmoid)
            nc.vector.tensor_tensor(sg[:, :ln], sg[:, :ln], pgs[i][:],
                                    _ALU.mult)
            nc.vector.tensor_tensor(dst[:, dst_blk, off:off + ln],
                                    sg[:, :ln], pus[i][:], _ALU.mult)

    def routing_block(j):
        """Router for compacted token block j: fp32 logits -> softmax ->
        group-limited top-2 -> this core's combine-weight col w_cap[:, j]."""
        hf_t = hfpool.tile([P, KO, P], _F32, tag="hf", name="hf_t")
        nc.sync.dma_start(hf_t[:], hfc_d[j])
        pl = mmp.tile([P, NEXP], _F32, tag="pu", bufs=4, name="pl")
        for ko in range(KO):
            nc.tensor.matmul(
                pl[:], hf_t[:, ko, :], gw8_sb[:, ko, :],
                start=(ko == 0), stop=(ko == KO - 1),
            )
        negmx = rpool.tile([P, 1], _F32, tag="negmx", name="negmx")
        nc.vector.tensor_reduce(negmx[:], pl[:], _X, _ALU.max, negate=True)
        ssum = rpool.tile([P, 1], _F32, tag="ssum", name="ssum")
        sc = rpool.tile([P, NEXP], _F32, tag="sc", name="sc")
        nc.scalar.activation(
            sc[:], pl[:], _ACTF.Exp, bias=negmx[:, 0:1], scale=1.0,
            accum_out=ssum[:, 0:1],
        )
        rec = rpool.tile([P, 1], _F32, tag="rec", name="rec")
        nc.vector.reciprocal(rec[:], ssum[:])
        sc2 = rpool.tile([P, NEXP], _F32, tag="sc2", name="sc2")
        nc.vector.tensor_scalar_mul(sc2[:], sc[:], rec[:, 0:1])
        # group scores: max over pairs of adjacent experts -> [P, 4]
        g = rpool.tile([P, 4], _F32, tag="g", name="g")
        nc.vector.tensor_reduce(
            g[:], sc2.rearrange("p (g e) -> p g e", e=2), _X, _ALU.max
        )
        m1g = rpool.tile([P, 1], _F32, tag="m1g", name="m1g")
        nc.vector.tensor_reduce(m1g[:], g[:], _X, _ALU.max)
        is1 = rpool.tile([P, 4], _F32, tag="is1", name="is1")
        nc.vector.tensor_scalar(is1[:], g[:], m1g[:, 0:1], None, _ALU.is_ge)
        gm = rpool.tile([P, 4], _F32, tag="gm", name="gm")
        nc.vector.scalar_tensor_tensor(
            gm[:], is1[:], -1e30, g[:], _ALU.mult, _ALU.add
        )
        m2g = rpool.tile([P, 1], _F32, tag="m2g", name="m2g")
        nc.vector.tensor_reduce(m2g[:], gm[:], _X, _ALU.max)
        gmask = rpool.tile([P, 4], _F32, tag="gmask", name="gmask")
        nc.vector.tensor_scalar(gmask[:], g[:], m2g[:, 0:1], None, _ALU.is_ge)
        smask = rpool.tile([P, NEXP], _F32, tag="smask", name="smask")
        sm_v = smask.rearrange("p (g e) -> p g e", e=2)
        nc.vector.tensor_copy(sm_v[:, :, 0], gmask[:])
        nc.vector.tensor_copy(sm_v[:, :, 1], gmask[:])
        msk = rpool.tile([P, NEXP], _F32, tag="msk", name="msk")
        nc.vector.tensor_tensor(msk[:], sc2[:], smask[:], _ALU.mult)
        m1e = rpool.tile([P, 1], _F32, tag="m1e", name="m1e")
        nc.vector.tensor_reduce(m1e[:], msk[:], _X, _ALU.max)
        is1e = rpool.tile([P, NEXP], _F32, tag="is1e", name="is1e")
        nc.vector.tensor_scalar(is1e[:], msk[:], m1e[:, 0:1], None, _ALU.is_ge)
        me = rpool.tile([P, NEXP], _F32, tag="me", name="me")
        nc.vector.scalar_tensor_tensor(
            me[:], is1e[:], -1e30, msk[:], _ALU.mult, _ALU.add
        )
        m2e = rpool.tile([P, 1], _F32, tag="m2e", name="m2e")
        nc.vector.tensor_reduce(m2e[:], me[:], _X, _ALU.max)
        wsel = rpool.tile([P, NEXP], _F32, tag="wsel", name="wsel")
        nc.vector.tensor_scalar(wsel[:], msk[:], m2e[:, 0:1], None, _ALU.is_ge)
        wall = rpool.tile([P, NEXP], _F32, tag="wall", name="wall")
        nc.vector.tensor_tensor(wall[:], msk[:], wsel[:], _ALU.mult)
        tmp8 = rpool.tile([P, NEXP], _F32, tag="tmp8", name="tmp8")
        nc.vector.tensor_tensor(tmp8[:], wall[:], esel_sb[:], _ALU.mult)
        nc.vector.tensor_reduce(w_cap[:, j : j + 1], tmp8[:], _X, _ALU.add)

    # ---- R phase: routing for this core's compacted tokens ----
    # (issued first so its small DMAs beat the big activation loads)
    for j in range(cb_n):
        routing_block(j)
    # ---- M phase: expert (sparse) + shared gate/up projections ----
    nc.sync.dma_start(hbe_sb[:], hbe_d[:])
    e_chunks = _chunks(capc)
    for fb in range(FBN):
        ffn_unit(gwl_d[fb], uwl_d[fb], aTe, fb, hbe_sb, e_chunks)
    nc.sync.dma_start(hbf_sb[:], hbf_d[:])
    s_chunks = _chunks(T)
    for sb in range(SBN):
        ffn_unit(sgl_d[sb], sul_d[sb], ash, sb, hbf_sb, s_chunks)
    # ---- D phase: down-projections, weight, write out ----
    e_blocks = _chunks(capc, P)
    for hb in range(HCH):
        dw_t = dpool.tile([P, FBN, 512], _BF16, tag="dw", name="dw_t")
        nc.sync.dma_start(dw_t[:], dwl_d[hb])
        sd_t = dpool.tile([P, SBN, 512], _BF16, tag="sd", name="sd_t")
        nc.sync.dma_start(sd_t[:], sdl_d[hb])
        for cb, (off, ln) in enumerate(e_blocks):
            cbs = slice(off, off + ln)
            pe = mmp.tile([P, 512], _F32, tag="pg", bufs=4, name="pe")
            for fb in range(FBN):
                nc.tensor.matmul(
                    pe[:ln, :], aTe[:, fb, cbs], dw_t[:, fb, :],
                    start=(fb == 0), stop=(fb == FBN - 1),
                )
            oe = opool.tile([P, 512], _F32, tag="oe", name="oe")
            nc.vector.tensor_scalar_mul(oe[:ln, :], pe[:ln, :],
                                        w_cap[:ln, cb : cb + 1])
            nc.sync.dma_start(oute_d[cbs, hb * 512:(hb + 1) * 512],
                              oe[:ln, :])
        for tb in range(TB):
            tbs = slice(tb * P, (tb + 1) * P)
            ps = mmp.tile([P, 512], _F32, tag="pu", bufs=4, name="ps")
            for sb in range(SBN):
                nc.tensor.matmul(
                    ps[:], ash[:, sb, tbs], sd_t[:, sb, :],
                    start=(sb == 0), stop=(sb == SBN - 1),
                )
            os_ = opool.tile([P, 512], _F32, tag="os", name="os_")
            nc.scalar.copy(os_[:], ps[:])
            nc.sync.dma_start(outs_d[tbs, hb * 512:(hb + 1) * 512], os_[:])


def build_program(cap=CAP0, capc=CAP0):
    nc = bacc.Bacc("TRN2", target_bir_lowering=False, debug=False)
    with tile.TileContext(nc) as tc:
        with ExitStack() as ctx:
            _build_body(ctx, tc, cap, capc)
    nc.compile()
    return nc


def _get_nc(cap=CAP0, capc=CAP0):
    if (cap, capc) not in _CACHED_NC:
        _CACHED_NC[(cap, capc)] = build_program(cap, capc)
    return _CACHED_NC[(cap, capc)]


def _host_route(h, gw):
    """Replicate the reference router's top-2 selection (fp32) to build
    the dispatch. Only token->expert ASSIGNMENT comes from here; the
    combine weights used in the output are computed on-device."""
    logits = (h @ gw.T).astype(F32)                       # [T, 8]
    m = logits.max(-1, keepdims=True)
    e = np.exp(logits - m, dtype=F32)
    sc = e / e.sum(-1, keepdims=True)
    gs = sc.reshape(-1, 4, 2).max(-1)                     # [T, 4]
    gidx = np.argsort(-gs, axis=1, kind="stable")[:, :2]
    gmask = np.zeros_like(gs)
    np.put_along_axis(gmask, gidx, 1.0, axis=1)
    smask = np.repeat(gmask, 2, axis=1)                   # [T, 8]
    masked = np.where(smask > 0, sc, 0.0)
    tidx = np.argsort(-masked, axis=1, kind="stable")[:, :2]
    return tidx


def make_in_maps(inputs, cap):
    """Host-side shard/layout prep: returns per-core input dicts and the
    per-expert (indices, count) used to unshard."""
    h = np.asarray(inputs["hidden_states"], F32).reshape(T, H)
    hT = np.ascontiguousarray(h.T)                              # [H, T]
    hbf_in = np.ascontiguousarray(
        hT.reshape(KO, P, T).transpose(1, 0, 2).astype(BF16)
    )
    gw = np.asarray(inputs["gate_weight"], F32)
    gw8T = gw.T                                                 # [H, 8]
    gw8_in = np.ascontiguousarray(gw8T.reshape(KO, P, NEXP).transpose(1, 0, 2))

    tidx = _host_route(h, gw)
    cb_n = cap // P

    gate_w = np.asarray(inputs["gate_w"], F32)
    up_w = np.asarray(inputs["up_w"], F32)
    down_w = np.asarray(inputs["down_w"], F32)
    sh_gate_w = np.asarray(inputs["sh_gate_w"], F32)
    sh_up_w = np.asarray(inputs["sh_up_w"], F32)
    sh_down_w = np.asarray(inputs["sh_down_w"], F32)

    in_maps, dispatch = [], []
    for n in range(NEXP):
        idx = np.nonzero((tidx == n).any(axis=1))[0]
        cnt = len(idx)
        assert cnt <= cap
        dispatch.append((idx, cnt))
        hTe = np.zeros((H, cap), F32)
        hTe[:, :cnt] = hT[:, idx]
        hbe_in = np.ascontiguousarray(
            hTe.reshape(KO, P, cap).transpose(1, 0, 2).astype(BF16)
        )
        hfc_in = np.ascontiguousarray(
            hTe.reshape(KO, P, cb_n, P).transpose(2, 1, 0, 3)
        )
        # expert weights: [fb, p(h-inner), ko(h-outer), f'] layouts
        gw4 = gate_w[n].reshape(FBN, P, KO, P)       # (fb, f', ko, p)
        gwl_in = np.ascontiguousarray(gw4.transpose(0, 3, 2, 1).astype(BF16))
        uw4 = up_w[n].reshape(FBN, P, KO, P)
        uwl_in = np.ascontiguousarray(uw4.transpose(0, 3, 2, 1).astype(BF16))
        # shared expert slice, padded 352 -> 384 channels
        shg = np.zeros((FSLP, H), F32)
        shg[:FSL] = sh_gate_w[n * FSL : (n + 1) * FSL]
        sgl_in = np.ascontiguousarray(
            shg.reshape(SBN, P, KO, P).transpose(0, 3, 2, 1).astype(BF16)
        )
        shu = np.zeros((FSLP, H), F32)
        shu[:FSL] = sh_up_w[n * FSL : (n + 1) * FSL]
        sul_in = np.ascontiguousarray(
            shu.reshape(SBN, P, KO, P).transpose(0, 3, 2, 1).astype(BF16)
        )
        # down weights: [hb, p(f-inner), fb, h'] layouts
        dw4 = down_w[n].reshape(HCH, 512, FBN, P)    # (hb, h', fb, f')
        dwl_in = np.ascontiguousarray(dw4.transpose(0, 3, 2, 1).astype(BF16))
        sd = np.zeros((H, FSLP), F32)
        sd[:, :FSL] = sh_down_w[:, n * FSL : (n + 1) * FSL]
        sdl_in = np.ascontiguousarray(
            sd.reshape(HCH, 512, SBN, P).transpose(0, 3, 2, 1).astype(BF16)
        )
        esel_in = np.zeros((P, NEXP), F32)
        esel_in[:, n] = 1.0
        in_maps.append({
            "hbf": hbf_in, "hbe": hbe_in, "hfc": hfc_in, "gw8": gw8_in,
            "esel": esel_in, "gwl": gwl_in, "uwl": uwl_in, "sgl": sgl_in,
            "sul": sul_in, "dwl": dwl_in, "sdl": sdl_in,
        })
    return in_maps, dispatch


def run(inputs, trace=False, **kwargs):
    h = np.asarray(inputs["hidden_states"], F32).reshape(T, H)
    tidx = _host_route(h, np.asarray(inputs["gate_weight"], F32))
    max_load = int(np.bincount(tidx.ravel(), minlength=NEXP).max())
    capc = -(-max_load // 64) * 64          # computed token columns
    cap = -(-capc // P) * P                 # padded layout capacity
    nc = _get_nc(cap, capc)
    in_maps, dispatch = make_in_maps(inputs, cap)
    res = run_bass_kernel_spmd(
        nc, in_maps, core_ids=list(range(NEXP)), trace=trace, **kwargs
    )
    total = res.results[0]["outs"].astype(F32)
    for i in range(1, NEXP):
        total = total + res.results[i]["outs"]
    for n in range(NEXP):
        idx, cnt = dispatch[n]
        total[idx] += res.results[n]["oute"][:cnt]
    return total.reshape(B, S, H), res


def kernel(**inputs):
    out, _ = run(inputs)
    return out


# revision 15
# speedup vs baseline: 2.1096x; 1.0354x over previous
"""DeepSeek-V2-style MoE kernel for 8 Trainium2 NeuronCores.

Sharding strategy (expert-parallel, SPARSE dispatch + shared-expert
channel-parallel):
  - The host replicates the router's top-2 selection (cheap [T,8] matmul
    in fp32 numpy) ONLY to build per-expert compacted token lists; every
    value that reaches the output is computed on-device, including the
    softmax/top-2 combine weights themselves.
  - Core n runs routed expert n on just the tokens routed to it (padded
    to a fixed capacity CAP, a multiple of 128), weighted by that
    expert's per-token combine weight computed on-device from fp32
    logits (group-limited top-2, same as dense baseline).
  - The always-on shared expert's intermediate dim FS=2816 is sharded
    8-ways (352 channels/core, padded to 384); each core's shared
    output over all T tokens is a partial sum.
  - Each core emits oute [CAP, H] (weighted expert rows, compacted
    order) and outs [T, H] (shared partial); the host sums the shared
    partials and scatter-adds the expert rows.

Heavy matmuls run in bf16 (fp32 PSUM accumulation). Weight-stationary
ko-outer loops amortize LDWEIGHTS over the token chunks; routing is
batched so the scalar engine's Exp table is loaded once.
"""

from contextlib import ExitStack

import numpy as np
import ml_dtypes

import concourse.bass as bass
import concourse.tile as tile
from concourse import bacc, mybir
from concourse.bass_utils import run_bass_kernel_spmd

BF16 = ml_dtypes.bfloat16
F32 = np.float32

P = 128
B, S, H, F, FS, NEXP = 2, 1024, 2048, 1408, 2816, 8
T = B * S                      # 2048 tokens
FSL = FS // NEXP               # 352 shared channels per core
FSLP = 384                     # padded to a multiple of 128
KO = H // P                    # 16 contraction chunks over H
TB = T // P                    # 16 token blocks of 128
TCH = T // 512                 # 4 token chunks of 512
HCH = H // 512                 # 4 output chunks of 512
FBN = F // P                   # 11 expert f-blocks
SBN = FSLP // P                # 3 shared fs-blocks
CAP0 = 640                     # default expert token capacity (5 blocks)

_X = mybir.AxisListType.X
_ALU = mybir.AluOpType
_ACTF = mybir.ActivationFunctionType
_F32 = mybir.dt.float32
_BF16 = mybir.dt.bfloat16

_CACHED_NC = {}


def _chunks(n, w=512):
    """Split n columns into <=w-wide matmul chunks."""
    out, off = [], 0
    while off < n:
        ln = min(w, n - off)
        out.append((off, ln))
        off += ln
    return out


def _build_body(ctx, tc, cap, capc):
    nc = tc.nc
    cb_n = cap // P
    hbf_d = nc.dram_tensor("hbf", [P, KO, T], _BF16, kind="ExternalInput").ap()
    hbe_d = nc.dram_tensor("hbe", [P, KO, cap], _BF16, kind="ExternalInput").ap()
    hfc_d = nc.dram_tensor("hfc", [cb_n, P, KO, P], _F32, kind="ExternalInput").ap()
    gw8_d = nc.dram_tensor("gw8", [P, KO, NEXP], _F32, kind="ExternalInput").ap()
    esel_d = nc.dram_tensor("esel", [P, NEXP], _F32, kind="ExternalInput").ap()
    gwl_d = nc.dram_tensor("gwl", [FBN, P, KO, P], _BF16, kind="ExternalInput").ap()
    uwl_d = nc.dram_tensor("uwl", [FBN, P, KO, P], _BF16, kind="ExternalInput").ap()
    sgl_d = nc.dram_tensor("sgl", [SBN, P, KO, P], _BF16, kind="ExternalInput").ap()
    sul_d = nc.dram_tensor("sul", [SBN, P, KO, P], _BF16, kind="ExternalInput").ap()
    dwl_d = nc.dram_tensor("dwl", [HCH, P, FBN, 512], _BF16, kind="ExternalInput").ap()
    sdl_d = nc.dram_tensor("sdl", [HCH, P, SBN, 512], _BF16, kind="ExternalInput").ap()
    oute_d = nc.dram_tensor("oute", [cap, H], _F32, kind="ExternalOutput").ap()
    outs_d = nc.dram_tensor("outs", [T, H], _F32, kind="ExternalOutput").ap()

    consts = ctx.enter_context(tc.tile_pool(name="consts", bufs=1))
    hbf_pool = ctx.enter_context(tc.tile_pool(name="hbfp", bufs=1))
    a_pool = ctx.enter_context(tc.tile_pool(name="apool", bufs=1))
    wpool = ctx.enter_context(tc.tile_pool(name="wpool", bufs=2))
    hfpool = ctx.enter_context(tc.tile_pool(name="hfpool", bufs=3))
    rpool = ctx.enter_context(tc.tile_pool(name="rpool", bufs=2))
    sgpool = ctx.enter_context(tc.tile_pool(name="sgpool", bufs=5))
    dpool = ctx.enter_context(tc.tile_pool(name="dpool", bufs=2))
    opool = ctx.enter_context(tc.tile_pool(name="opool", bufs=3))
    mmp = ctx.enter_context(tc.tile_pool(name="mmp", bufs=1, space="PSUM"))

    gw8_sb = consts.tile([P, KO, NEXP], _F32)
    nc.sync.dma_start(gw8_sb[:], gw8_d[:])
    esel_sb = consts.tile([P, NEXP], _F32)
    nc.sync.dma_start(esel_sb[:], esel_d[:])
    w_cap = consts.tile([P, cb_n], _F32)

    hbf_sb = hbf_pool.tile([P, KO, T], _BF16)
    hbe_sb = hbf_pool.tile([P, KO, cap], _BF16)

    aTe = a_pool.tile([P, FBN, capc], _BF16)
    ash = a_pool.tile([P, SBN, T], _BF16)

    def ffn_unit(wg_src, wu_src, dst, dst_blk, src_sb, chunks):
        """Gate/up matmuls + silu*up for one 128-wide block of the
        intermediate dim. ko-outer: each LDWEIGHTS feeds len(chunks)
        matmuls."""
        # weight DMAs dispatch from the Activation hwdge queue so their
        # pool-slot gating never stalls the bulk stream on the SP queue
        wg_t = wpool.tile([P, KO, P], _BF16, tag="wg", name="wg_t")
        nc.scalar.dma_start(wg_t[:], wg_src)
        wu_t = wpool.tile([P, KO, P], _BF16, tag="wu", name="wu_t")
        nc.scalar.dma_start(wu_t[:], wu_src)
        pgs = [mmp.tile([P, ln], _F32, tag="pg", bufs=4, name=f"pg{i}")
               for i, (_, ln) in enumerate(chunks)]
        for ko in range(KO):
            for i, (off, ln) in enumerate(chunks):
                nc.tensor.matmul(
                    pgs[i][:], wg_t[:, ko, :],
                    src_sb[:, ko, off:off + ln],
                    start=(ko == 0), stop=(ko == KO - 1),
                )
        pus = [mmp.tile([P, ln], _F32, tag="pu", bufs=4, name=f"pu{i}")
               for i, (_, ln) in enumerate(chunks)]
        for ko in range(KO):
            for i, (off, ln) in enumerate(chunks):
                nc.tensor.matmul(
                    pus[i][:], wu_t[:, ko, :],
                    src_sb[:, ko, off:off + ln],
                    start=(ko == 0), stop=(ko == KO - 1),
                )
        for i, (off, ln) in enumerate(chunks):
            sg = sgpool.tile([P, 512], _F32, tag="sg", name="sg")
            nc.scalar.activation(sg[:, :ln], pgs[i][:], _ACTF.Sigmoid)
            nc.vector.tensor_tensor(sg[:, :ln], sg[:, :ln], pgs[i][:],
                                    _ALU.mult)
            nc.vector.tensor_tensor(dst[:, dst_blk, off:off + ln],
                                    sg[:, :ln], pus[i][:], _ALU.mult)

    def routing_block(j):
        """Router for compacted token block j: fp32 logits -> softmax ->
        group-limited top-2 -> this core's combine-weight col w_cap[:, j]."""
        hf_t = hfpool.tile([P, KO, P], _F32, tag="hf", name="hf_t")
        nc.sync.dma_start(hf_t[:], hfc_d[j])
        pl = mmp.tile([P, NEXP], _F32, tag="pu", bufs=4, name="pl")
        for ko in range(KO):
            nc.tensor.matmul(
                pl[:], hf_t[:, ko, :], gw8_sb[:, ko, :],
                start=(ko == 0), stop=(ko == KO - 1),
            )
        negmx = rpool.tile([P, 1], _F32, tag="negmx", name="negmx")
        nc.vector.tensor_reduce(negmx[:], pl[:], _X, _ALU.max, negate=True)
        ssum = rpool.tile([P, 1], _F32, tag="ssum", name="ssum")
        sc = rpool.tile([P, NEXP], _F32, tag="sc", name="sc")
        nc.scalar.activation(
            sc[:], pl[:], _ACTF.Exp, bias=negmx[:, 0:1], scale=1.0,
            accum_out=ssum[:, 0:1],
        )
        rec = rpool.tile([P, 1], _F32, tag="rec", name="rec")
        nc.vector.reciprocal(rec[:], ssum[:])
        sc2 = rpool.tile([P, NEXP], _F32, tag="sc2", name="sc2")
        nc.vector.tensor_scalar_mul(sc2[:], sc[:], rec[:, 0:1])
        # group scores: max over pairs of adjacent experts -> [P, 4]
        g = rpool.tile([P, 4], _F32, tag="g", name="g")
        nc.vector.tensor_reduce(
            g[:], sc2.rearrange("p (g e) -> p g e", e=2), _X, _ALU.max
        )
        m1g = rpool.tile([P, 1], _F32, tag="m1g", name="m1g")
        nc.vector.tensor_reduce(m1g[:], g[:], _X, _ALU.max)
        is1 = rpool.tile([P, 4], _F32, tag="is1", name="is1")
        nc.vector.tensor_scalar(is1[:], g[:], m1g[:, 0:1], None, _ALU.is_ge)
        gm = rpool.tile([P, 4], _F32, tag="gm", name="gm")
        nc.vector.scalar_tensor_tensor(
            gm[:], is1[:], -1e30, g[:], _ALU.mult, _ALU.add
        )
        m2g = rpool.tile([P, 1], _F32, tag="m2g", name="m2g")
        nc.vector.tensor_reduce(m2g[:], gm[:], _X, _ALU.max)
        gmask = rpool.tile([P, 4], _F32, tag="gmask", name="gmask")
        nc.vector.tensor_scalar(gmask[:], g[:], m2g[:, 0:1], None, _ALU.is_ge)
        smask = rpool.tile([P, NEXP], _F32, tag="smask", name="smask")
        sm_v = smask.rearrange("p (g e) -> p g e", e=2)
        nc.vector.tensor_copy(sm_v[:, :, 0], gmask[:])
        nc.vector.tensor_copy(sm_v[:, :, 1], gmask[:])
        msk = rpool.tile([P, NEXP], _F32, tag="msk", name="msk")
        nc.vector.tensor_tensor(msk[:], sc2[:], smask[:], _ALU.mult)
        m1e = rpool.tile([P, 1], _F32, tag="m1e", name="m1e")
        nc.vector.tensor_reduce(m1e[:], msk[:], _X, _ALU.max)
        is1e = rpool.tile([P, NEXP], _F32, tag="is1e", name="is1e")
        nc.vector.tensor_scalar(is1e[:], msk[:], m1e[:, 0:1], None, _ALU.is_ge)
        me = rpool.tile([P, NEXP], _F32, tag="me", name="me")
        nc.vector.scalar_tensor_tensor(
            me[:], is1e[:], -1e30, msk[:], _ALU.mult, _ALU.add
        )
        m2e = rpool.tile([P, 1], _F32, tag="m2e", name="m2e")
        nc.vector.tensor_reduce(m2e[:], me[:], _X, _ALU.max)
        wsel = rpool.tile([P, NEXP], _F32, tag="wsel", name="wsel")
        nc.vector.tensor_scalar(wsel[:], msk[:], m2e[:, 0:1], None, _ALU.is_ge)
        wall = rpool.tile([P, NEXP], _F32, tag="wall", name="wall")
        nc.vector.tensor_tensor(wall[:], msk[:], wsel[:], _ALU.mult)
        tmp8 = rpool.tile([P, NEXP], _F32, tag="tmp8", name="tmp8")
        nc.vector.tensor_tensor(tmp8[:], wall[:], esel_sb[:], _ALU.mult)
        nc.vector.tensor_reduce(w_cap[:, j : j + 1], tmp8[:], _X, _ALU.add)

    # ---- R + M-expert, interleaved ----
    # First 3 routing blocks run up front (their hfc DMAs are ungated with
    # hfpool bufs=3); the rest slot in after two expert units so their
    # slot-gated hfc DMAs never stall the in-order tensor queue.
    r_head = min(3, cb_n)
    for j in range(r_head):
        routing_block(j)
    nc.sync.dma_start(hbe_sb[:], hbe_d[:])
    e_chunks = _chunks(capc)
    for fb in range(min(2, FBN)):
        ffn_unit(gwl_d[fb], uwl_d[fb], aTe, fb, hbe_sb, e_chunks)
    for j in range(r_head, cb_n):
        routing_block(j)
    for fb in range(2, FBN):
        ffn_unit(gwl_d[fb], uwl_d[fb], aTe, fb, hbe_sb, e_chunks)
    nc.sync.dma_start(hbf_sb[:], hbf_d[:])
    s_chunks = _chunks(T)
    for sb in range(SBN):
        ffn_unit(sgl_d[sb], sul_d[sb], ash, sb, hbf_sb, s_chunks)
    # ---- D phase: down-projections, weight, write out ----
    e_blocks = _chunks(capc, P)
    for hb in range(HCH):
        dw_t = dpool.tile([P, FBN, 512], _BF16, tag="dw", name="dw_t")
        nc.sync.dma_start(dw_t[:], dwl_d[hb])
        sd_t = dpool.tile([P, SBN, 512], _BF16, tag="sd", name="sd_t")
        nc.sync.dma_start(sd_t[:], sdl_d[hb])
        for cb, (off, ln) in enumerate(e_blocks):
            cbs = slice(off, off + ln)
            pe = mmp.tile([P, 512], _F32, tag="pg", bufs=4, name="pe")
            for fb in range(FBN):
                nc.tensor.matmul(
                    pe[:ln, :], aTe[:, fb, cbs], dw_t[:, fb, :],
                    start=(fb == 0), stop=(fb == FBN - 1),
                )
            oe = opool.tile([P, 512], _F32, tag="oe", name="oe")
            nc.vector.tensor_scalar_mul(oe[:ln, :], pe[:ln, :],
                                        w_cap[:ln, cb : cb + 1])
            nc.scalar.dma_start(oute_d[cbs, hb * 512:(hb + 1) * 512],
                                oe[:ln, :])
        for tb in range(TB):
            tbs = slice(tb * P, (tb + 1) * P)
            ps = mmp.tile([P, 512], _F32, tag="pu", bufs=4, name="ps")
            for sb in range(SBN):
                nc.tensor.matmul(
                    ps[:], ash[:, sb, tbs], sd_t[:, sb, :],
                    start=(sb == 0), stop=(sb == SBN - 1),
                )
            os_ = opool.tile([P, 512], _F32, tag="os", name="os_")
            nc.vector.tensor_copy(os_[:], ps[:])
            nc.scalar.dma_start(outs_d[tbs, hb * 512:(hb + 1) * 512], os_[:])


def build_program(cap=CAP0, capc=CAP0):
    nc = bacc.Bacc("TRN2", target_bir_lowering=False, debug=False)
    with tile.TileContext(nc) as tc:
        with ExitStack() as ctx:
            _build_body(ctx, tc, cap, capc)
    nc.compile()
    return nc


def _get_nc(cap=CAP0, capc=CAP0):
    if (cap, capc) not in _CACHED_NC:
        _CACHED_NC[(cap, capc)] = build_program(cap, capc)
    return _CACHED_NC[(cap, capc)]


def _host_route(h, gw):
    """Replicate the reference router's top-2 selection (fp32) to build
    the dispatch. Only token->expert ASSIGNMENT comes from here; the
    combine weights used in the output are computed on-device."""
    logits = (h @ gw.T).astype(F32)                       # [T, 8]
    m = logits.max(-1, keepdims=True)
    e = np.exp(logits - m, dtype=F32)
    sc = e / e.sum(-1, keepdims=True)
    gs = sc.reshape(-1, 4, 2).max(-1)                     # [T, 4]
    gidx = np.argsort(-gs, axis=1, kind="stable")[:, :2]
    gmask = np.zeros_like(gs)
    np.put_along_axis(gmask, gidx, 1.0, axis=1)
    smask = np.repeat(gmask, 2, axis=1)                   # [T, 8]
    masked = np.where(smask > 0, sc, 0.0)
    tidx = np.argsort(-masked, axis=1, kind="stable")[:, :2]
    return tidx


def make_in_maps(inputs, cap):
    """Host-side shard/layout prep: returns per-core input dicts and the
    per-expert (indices, count) used to unshard."""
    h = np.asarray(inputs["hidden_states"], F32).reshape(T, H)
    hT = np.ascontiguousarray(h.T)                              # [H, T]
    hbf_in = np.ascontiguousarray(
        hT.reshape(KO, P, T).transpose(1, 0, 2).astype(BF16)
    )
    gw = np.asarray(inputs["gate_weight"], F32)
    gw8T = gw.T                                                 # [H, 8]
    gw8_in = np.ascontiguousarray(gw8T.reshape(KO, P, NEXP).transpose(1, 0, 2))

    tidx = _host_route(h, gw)
    cb_n = cap // P

    gate_w = np.asarray(inputs["gate_w"], F32)
    up_w = np.asarray(inputs["up_w"], F32)
    down_w = np.asarray(inputs["down_w"], F32)
    sh_gate_w = np.asarray(inputs["sh_gate_w"], F32)
    sh_up_w = np.asarray(inputs["sh_up_w"], F32)
    sh_down_w = np.asarray(inputs["sh_down_w"], F32)

    in_maps, dispatch = [], []
    for n in range(NEXP):
        idx = np.nonzero((tidx == n).any(axis=1))[0]
        cnt = len(idx)
        assert cnt <= cap
        dispatch.append((idx, cnt))
        hTe = np.zeros((H, cap), F32)
        hTe[:, :cnt] = hT[:, idx]
        hbe_in = np.ascontiguousarray(
            hTe.reshape(KO, P, cap).transpose(1, 0, 2).astype(BF16)
        )
        hfc_in = np.ascontiguousarray(
            hTe.reshape(KO, P, cb_n, P).transpose(2, 1, 0, 3)
        )
        # expert weights: [fb, p(h-inner), ko(h-outer), f'] layouts
        gw4 = gate_w[n].reshape(FBN, P, KO, P)       # (fb, f', ko, p)
        gwl_in = np.ascontiguousarray(gw4.transpose(0, 3, 2, 1).astype(BF16))
        uw4 = up_w[n].reshape(FBN, P, KO, P)
        uwl_in = np.ascontiguousarray(uw4.transpose(0, 3, 2, 1).astype(BF16))
        # shared expert slice, padded 352 -> 384 channels
        shg = np.zeros((FSLP, H), F32)
        shg[:FSL] = sh_gate_w[n * FSL : (n + 1) * FSL]
        sgl_in = np.ascontiguousarray(
            shg.reshape(SBN, P, KO, P).transpose(0, 3, 2, 1).astype(BF16)
        )
        shu = np.zeros((FSLP, H), F32)
        shu[:FSL] = sh_up_w[n * FSL : (n + 1) * FSL]
        sul_in = np.ascontiguousarray(
            shu.reshape(SBN, P, KO, P).transpose(0, 3, 2, 1).astype(BF16)
        )
        # down weights: [hb, p(f-inner), fb, h'] layouts
        dw4 = down_w[n].reshape(HCH, 512, FBN, P)    # (hb, h', fb, f')
        dwl_in = np.ascontiguousarray(dw4.transpose(0, 3, 2, 1).astype(BF16))
        sd = np.zeros((H, FSLP), F32)
        sd[:, :FSL] = sh_down_w[:, n * FSL : (n + 1) * FSL]
        sdl_in = np.ascontiguousarray(
            sd.reshape(HCH, 512, SBN, P).transpose(0, 3, 2, 1).astype(BF16)
        )
        esel_in = np.zeros((P, NEXP), F32)
        esel_in[:, n] = 1.0
        in_maps.append({
            "hbf": hbf_in, "hbe": hbe_in, "hfc": hfc_in, "gw8": gw8_in,
            "esel": esel_in, "gwl": gwl_in, "uwl": uwl_in, "sgl": sgl_in,
            "sul": sul_in, "dwl": dwl_in, "sdl": sdl_in,
        })
    return in_maps, dispatch


def run(inputs, trace=False, **kwargs):
    h = np.asarray(inputs["hidden_states"], F32).reshape(T, H)
    tidx = _host_route(h, np.asarray(inputs["gate_weight"], F32))
    max_load = int(np.bincount(tidx.ravel(), minlength=NEXP).max())
    capc = -(-max_load // 64) * 64          # computed token columns
    cap = -(-capc // P) * P                 # padded layout capacity
    nc = _get_nc(cap, capc)
    in_maps, dispatch = make_in_maps(inputs, cap)
    res = run_bass_kernel_spmd(
        nc, in_maps, core_ids=list(range(NEXP)), trace=trace, **kwargs
    )
    total = res.results[0]["outs"].astype(F32)
    for i in range(1, NEXP):
        total = total + res.results[i]["outs"]
    for n in range(NEXP):
        idx, cnt = dispatch[n]
        total[idx] += res.results[n]["oute"][:cnt]
    return total.reshape(B, S, H), res


def kernel(**inputs):
    out, _ = run(inputs)
    return out


# revision 18
# speedup vs baseline: 2.1396x; 1.0142x over previous
"""DeepSeek-V2-style MoE kernel for 8 Trainium2 NeuronCores.

Sharding strategy (expert-parallel, SPARSE dispatch + shared-expert
channel-parallel):
  - The host replicates the router's top-2 selection (cheap [T,8] matmul
    in fp32 numpy) ONLY to build per-expert compacted token lists; every
    value that reaches the output is computed on-device, including the
    softmax/top-2 combine weights themselves.
  - Core n runs routed expert n on just the tokens routed to it (padded
    to a fixed capacity CAP, a multiple of 128), weighted by that
    expert's per-token combine weight computed on-device from fp32
    logits (group-limited top-2, same as dense baseline).
  - The always-on shared expert's intermediate dim FS=2816 is sharded
    8-ways (352 channels/core, padded to 384); each core's shared
    output over all T tokens is a partial sum.
  - Each core emits oute [CAP, H] (weighted expert rows, compacted
    order) and outs [T, H] (shared partial); the host sums the shared
    partials and scatter-adds the expert rows.

Heavy matmuls run in bf16 (fp32 PSUM accumulation). Weight-stationary
ko-outer loops amortize LDWEIGHTS over the token chunks; routing is
batched so the scalar engine's Exp table is loaded once.
"""

from contextlib import ExitStack

import numpy as np
import ml_dtypes

import concourse.bass as bass
import concourse.tile as tile
from concourse import bacc, mybir
from concourse.bass_utils import run_bass_kernel_spmd

BF16 = ml_dtypes.bfloat16
F32 = np.float32

P = 128
B, S, H, F, FS, NEXP = 2, 1024, 2048, 1408, 2816, 8
T = B * S                      # 2048 tokens
FSL = FS // NEXP               # 352 shared channels per core
FSLP = 384                     # padded to a multiple of 128
KO = H // P                    # 16 contraction chunks over H
TB = T // P                    # 16 token blocks of 128
TCH = T // 512                 # 4 token chunks of 512
HCH = H // 512                 # 4 output chunks of 512
FBN = F // P                   # 11 expert f-blocks
SBN = FSLP // P                # 3 shared fs-blocks
CAP0 = 640                     # default expert token capacity (5 blocks)

_X = mybir.AxisListType.X
_ALU = mybir.AluOpType
_ACTF = mybir.ActivationFunctionType
_F32 = mybir.dt.float32
_BF16 = mybir.dt.bfloat16

_CACHED_NC = {}


def _chunks(n, w=512):
    """Split n columns into <=w-wide matmul chunks."""
    out, off = [], 0
    while off < n:
        ln = min(w, n - off)
        out.append((off, ln))
        off += ln
    return out


def _build_body(ctx, tc, cap, capc):
    nc = tc.nc
    cb_n = cap // P
    hbf_d = nc.dram_tensor("hbf", [P, KO, T], _BF16, kind="ExternalInput").ap()
    hbe_d = nc.dram_tensor("hbe", [P, KO, cap], _BF16, kind="ExternalInput").ap()
    hfc_d = nc.dram_tensor("hfc", [cb_n, P, KO, P], _F32, kind="ExternalInput").ap()
    gw8_d = nc.dram_tensor("gw8", [P, KO, NEXP], _F32, kind="ExternalInput").ap()
    esel_d = nc.dram_tensor("esel", [P, NEXP], _F32, kind="ExternalInput").ap()
    gwl_d = nc.dram_tensor("gwl", [FBN, P, KO, P], _BF16, kind="ExternalInput").ap()
    uwl_d = nc.dram_tensor("uwl", [FBN, P, KO, P], _BF16, kind="ExternalInput").ap()
    sgl_d = nc.dram_tensor("sgl", [SBN, P, KO, P], _BF16, kind="ExternalInput").ap()
    sul_d = nc.dram_tensor("sul", [SBN, P, KO, P], _BF16, kind="ExternalInput").ap()
    dwl_d = nc.dram_tensor("dwl", [HCH, P, FBN, 512], _BF16, kind="ExternalInput").ap()
    sdl_d = nc.dram_tensor("sdl", [HCH, P, SBN, 512], _BF16, kind="ExternalInput").ap()
    oute_d = nc.dram_tensor("oute", [cap, H], _F32, kind="ExternalOutput").ap()
    outs_d = nc.dram_tensor("outs", [T, H], _F32, kind="ExternalOutput").ap()

    consts = ctx.enter_context(tc.tile_pool(name="consts", bufs=1))
    hbf_pool = ctx.enter_context(tc.tile_pool(name="hbfp", bufs=1))
    a_pool = ctx.enter_context(tc.tile_pool(name="apool", bufs=1))
    wpool = ctx.enter_context(tc.tile_pool(name="wpool", bufs=2))
    hfpool = ctx.enter_context(tc.tile_pool(name="hfpool", bufs=3))
    rpool = ctx.enter_context(tc.tile_pool(name="rpool", bufs=2))
    sgpool = ctx.enter_context(tc.tile_pool(name="sgpool", bufs=3))
    dpool = ctx.enter_context(tc.tile_pool(name="dpool", bufs=2))
    opool = ctx.enter_context(tc.tile_pool(name="opool", bufs=3))
    mmp = ctx.enter_context(tc.tile_pool(name="mmp", bufs=1, space="PSUM"))

    # consts go on the Activation queue AHEAD of the weight stream so the
    # first routing block's inputs are not bandwidth-starved by weights
    gw8_sb = consts.tile([P, KO, NEXP], _F32)
    nc.scalar.dma_start(gw8_sb[:], gw8_d[:])
    esel_sb = consts.tile([P, NEXP], _F32)
    nc.scalar.dma_start(esel_sb[:], esel_d[:])
    w_cap = consts.tile([P, cb_n], _F32)

    hbf_sb = hbf_pool.tile([P, KO, T], _BF16)
    hbe_sb = hbf_pool.tile([P, KO, cap], _BF16)

    aTe = a_pool.tile([P, FBN, capc], _BF16)
    ash = a_pool.tile([P, SBN, T], _BF16)

    def ffn_unit(wg_src, wu_src, dst, dst_blk, src_sb, chunks):
        """Gate/up matmuls + silu*up for one 128-wide block of the
        intermediate dim. ko-outer: each LDWEIGHTS feeds len(chunks)
        matmuls."""
        # weight DMAs dispatch from the Activation hwdge queue so their
        # pool-slot gating never stalls the bulk stream on the SP queue
        wg_t = wpool.tile([P, KO, P], _BF16, tag="wg", name="wg_t")
        nc.scalar.dma_start(wg_t[:], wg_src)
        wu_t = wpool.tile([P, KO, P], _BF16, tag="wu", name="wu_t")
        nc.scalar.dma_start(wu_t[:], wu_src)
        pgs = [mmp.tile([P, ln], _F32, tag="pg", bufs=4, name=f"pg{i}")
               for i, (_, ln) in enumerate(chunks)]
        for ko in range(KO):
            for i, (off, ln) in enumerate(chunks):
                nc.tensor.matmul(
                    pgs[i][:], wg_t[:, ko, :],
                    src_sb[:, ko, off:off + ln],
                    start=(ko == 0), stop=(ko == KO - 1),
                )
        pus = [mmp.tile([P, ln], _F32, tag="pu", bufs=4, name=f"pu{i}")
               for i, (_, ln) in enumerate(chunks)]
        for ko in range(KO):
            for i, (off, ln) in enumerate(chunks):
                nc.tensor.matmul(
                    pus[i][:], wu_t[:, ko, :],
                    src_sb[:, ko, off:off + ln],
                    start=(ko == 0), stop=(ko == KO - 1),
                )
        for i, (off, ln) in enumerate(chunks):
            sg = sgpool.tile([P, 512], _F32, tag="sg", name="sg")
            nc.scalar.activation(sg[:, :ln], pgs[i][:], _ACTF.Sigmoid)
            nc.vector.tensor_tensor(sg[:, :ln], sg[:, :ln], pgs[i][:],
                                    _ALU.mult)
            nc.vector.tensor_tensor(dst[:, dst_blk, off:off + ln],
                                    sg[:, :ln], pus[i][:], _ALU.mult)

    def routing_block(j):
        """Router for compacted token block j: fp32 logits -> softmax ->
        group-limited top-2 -> this core's combine-weight col w_cap[:, j]."""
        hf_t = hfpool.tile([P, KO, P], _F32, tag="hf", name="hf_t")
        eng = nc.scalar if j == 0 else nc.sync
        eng.dma_start(hf_t[:], hfc_d[j])
        pl = mmp.tile([P, NEXP], _F32, tag="pu", bufs=4, name="pl")
        for ko in range(KO):
            nc.tensor.matmul(
                pl[:], hf_t[:, ko, :], gw8_sb[:, ko, :],
                start=(ko == 0), stop=(ko == KO - 1),
            )
        negmx = rpool.tile([P, 1], _F32, tag="negmx", name="negmx")
        nc.vector.tensor_reduce(negmx[:], pl[:], _X, _ALU.max, negate=True)
        ssum = rpool.tile([P, 1], _F32, tag="ssum", name="ssum")
        sc = rpool.tile([P, NEXP], _F32, tag="sc", name="sc")
        nc.scalar.activation(
            sc[:], pl[:], _ACTF.Exp, bias=negmx[:, 0:1], scale=1.0,
            accum_out=ssum[:, 0:1],
        )
        rec = rpool.tile([P, 1], _F32, tag="rec", name="rec")
        nc.vector.reciprocal(rec[:], ssum[:])
        sc2 = rpool.tile([P, NEXP], _F32, tag="sc2", name="sc2")
        nc.vector.tensor_scalar_mul(sc2[:], sc[:], rec[:, 0:1])
        # group scores: max over pairs of adjacent experts -> [P, 4]
        g = rpool.tile([P, 4], _F32, tag="g", name="g")
        nc.vector.tensor_reduce(
            g[:], sc2.rearrange("p (g e) -> p g e", e=2), _X, _ALU.max
        )
        m1g = rpool.tile([P, 1], _F32, tag="m1g", name="m1g")
        nc.vector.tensor_reduce(m1g[:], g[:], _X, _ALU.max)
        is1 = rpool.tile([P, 4], _F32, tag="is1", name="is1")
        nc.vector.tensor_scalar(is1[:], g[:], m1g[:, 0:1], None, _ALU.is_ge)
        gm = rpool.tile([P, 4], _F32, tag="gm", name="gm")
        nc.vector.scalar_tensor_tensor(
            gm[:], is1[:], -1e30, g[:], _ALU.mult, _ALU.add
        )
        m2g = rpool.tile([P, 1], _F32, tag="m2g", name="m2g")
        nc.vector.tensor_reduce(m2g[:], gm[:], _X, _ALU.max)
        gmask = rpool.tile([P, 4], _F32, tag="gmask", name="gmask")
        nc.vector.tensor_scalar(gmask[:], g[:], m2g[:, 0:1], None, _ALU.is_ge)
        smask = rpool.tile([P, NEXP], _F32, tag="smask", name="smask")
        sm_v = smask.rearrange("p (g e) -> p g e", e=2)
        nc.vector.tensor_copy(sm_v[:, :, 0], gmask[:])
        nc.vector.tensor_copy(sm_v[:, :, 1], gmask[:])
        msk = rpool.tile([P, NEXP], _F32, tag="msk", name="msk")
        nc.vector.tensor_tensor(msk[:], sc2[:], smask[:], _ALU.mult)
        m1e = rpool.tile([P, 1], _F32, tag="m1e", name="m1e")
        nc.vector.tensor_reduce(m1e[:], msk[:], _X, _ALU.max)
        is1e = rpool.tile([P, NEXP], _F32, tag="is1e", name="is1e")
        nc.vector.tensor_scalar(is1e[:], msk[:], m1e[:, 0:1], None, _ALU.is_ge)
        me = rpool.tile([P, NEXP], _F32, tag="me", name="me")
        nc.vector.scalar_tensor_tensor(
            me[:], is1e[:], -1e30, msk[:], _ALU.mult, _ALU.add
        )
        m2e = rpool.tile([P, 1], _F32, tag="m2e", name="m2e")
        nc.vector.tensor_reduce(m2e[:], me[:], _X, _ALU.max)
        wsel = rpool.tile([P, NEXP], _F32, tag="wsel", name="wsel")
        nc.vector.tensor_scalar(wsel[:], msk[:], m2e[:, 0:1], None, _ALU.is_ge)
        wall = rpool.tile([P, NEXP], _F32, tag="wall", name="wall")
        nc.vector.tensor_tensor(wall[:], msk[:], wsel[:], _ALU.mult)
        tmp8 = rpool.tile([P, NEXP], _F32, tag="tmp8", name="tmp8")
        nc.vector.tensor_tensor(tmp8[:], wall[:], esel_sb[:], _ALU.mult)
        nc.vector.tensor_reduce(w_cap[:, j : j + 1], tmp8[:], _X, _ALU.add)

    # ---- R + M-expert, interleaved ----
    # First 3 routing blocks run up front (their hfc DMAs are ungated with
    # hfpool bufs=3); the rest slot in after two expert units so their
    # slot-gated hfc DMAs never stall the in-order tensor queue.
    r_head = min(3, cb_n)
    for j in range(r_head):
        routing_block(j)
    nc.sync.dma_start(hbe_sb[:], hbe_d[:])
    e_chunks = _chunks(capc)
    for fb in range(min(2, FBN)):
        ffn_unit(gwl_d[fb], uwl_d[fb], aTe, fb, hbe_sb, e_chunks)
    for j in range(r_head, cb_n):
        routing_block(j)
    for fb in range(2, FBN):
        ffn_unit(gwl_d[fb], uwl_d[fb], aTe, fb, hbe_sb, e_chunks)
    nc.sync.dma_start(hbf_sb[:], hbf_d[:])
    s_chunks = _chunks(T)
    for sb in range(SBN):
        ffn_unit(sgl_d[sb], sul_d[sb], ash, sb, hbf_sb, s_chunks)
    # ---- D phase: down-projections, weight, write out ----
    e_blocks = _chunks(capc, P)
    for hb in range(HCH):
        sd_t = dpool.tile([P, SBN, 512], _BF16, tag="sd", name="sd_t")
        nc.sync.dma_start(sd_t[:], sdl_d[hb])
        dw_t = dpool.tile([P, FBN, 512], _BF16, tag="dw", name="dw_t",
                          bufs=3)
        nc.sync.dma_start(dw_t[:], dwl_d[hb])
        # shared first: its long copy+DMA consumer chain drains during the
        # expert blocks, shortening the kernel tail
        for tb in range(TB):
            tbs = slice(tb * P, (tb + 1) * P)
            ps = mmp.tile([P, 512], _F32, tag="pu", bufs=4, name="ps")
            for sb in range(SBN):
                nc.tensor.matmul(
                    ps[:], ash[:, sb, tbs], sd_t[:, sb, :],
                    start=(sb == 0), stop=(sb == SBN - 1),
                )
            os_ = opool.tile([P, 512], _F32, tag="os", name="os_")
            nc.vector.tensor_copy(os_[:], ps[:])
            eng = nc.scalar if tb % 2 else nc.sync
            eng.dma_start(outs_d[tbs, hb * 512:(hb + 1) * 512], os_[:])
        for cb, (off, ln) in enumerate(e_blocks):
            cbs = slice(off, off + ln)
            pe = mmp.tile([P, 512], _F32, tag="pg", bufs=4, name="pe")
            for fb in range(FBN):
                nc.tensor.matmul(
                    pe[:ln, :], aTe[:, fb, cbs], dw_t[:, fb, :],
                    start=(fb == 0), stop=(fb == FBN - 1),
                )
            oe = opool.tile([P, 512], _F32, tag="oe", name="oe", bufs=2)
            nc.vector.tensor_scalar_mul(oe[:ln, :], pe[:ln, :],
                                        w_cap[:ln, cb : cb + 1])
            eng = nc.scalar if cb % 2 else nc.sync
            eng.dma_start(oute_d[cbs, hb * 512:(hb + 1) * 512], oe[:ln, :])


def build_program(cap=CAP0, capc=CAP0):
    nc = bacc.Bacc("TRN2", target_bir_lowering=False, debug=False)
    with tile.TileContext(nc) as tc:
        with ExitStack() as ctx:
            _build_body(ctx, tc, cap, capc)
    nc.compile()
    return nc


def _get_nc(cap=CAP0, capc=CAP0):
    if (cap, capc) not in _CACHED_NC:
        _CACHED_NC[(cap, capc)] = build_program(cap, capc)
    return _CACHED_NC[(cap, capc)]


def _host_route(h, gw):
    """Replicate the reference router's top-2 selection (fp32) to build
    the dispatch. Only token->expert ASSIGNMENT comes from here; the
    combine weights used in the output are computed on-device."""
    logits = (h @ gw.T).astype(F32)                       # [T, 8]
    m = logits.max(-1, keepdims=True)
    e = np.exp(logits - m, dtype=F32)
    sc = e / e.sum(-1, keepdims=True)
    gs = sc.reshape(-1, 4, 2).max(-1)                     # [T, 4]
    gidx = np.argsort(-gs, axis=1, kind="stable")[:, :2]
    gmask = np.zeros_like(gs)
    np.put_along_axis(gmask, gidx, 1.0, axis=1)
    smask = np.repeat(gmask, 2, axis=1)                   # [T, 8]
    masked = np.where(smask > 0, sc, 0.0)
    tidx = np.argsort(-masked, axis=1, kind="stable")[:, :2]
    return tidx


def make_in_maps(inputs, cap):
    """Host-side shard/layout prep: returns per-core input dicts and the
    per-expert (indices, count) used to unshard."""
    h = np.asarray(inputs["hidden_states"], F32).reshape(T, H)
    hT = np.ascontiguousarray(h.T)                              # [H, T]
    hbf_in = np.ascontiguousarray(
        hT.reshape(KO, P, T).transpose(1, 0, 2).astype(BF16)
    )
    gw = np.asarray(inputs["gate_weight"], F32)
    gw8T = gw.T                                                 # [H, 8]
    gw8_in = np.ascontiguousarray(gw8T.reshape(KO, P, NEXP).transpose(1, 0, 2))

    tidx = _host_route(h, gw)
    cb_n = cap // P

    gate_w = np.asarray(inputs["gate_w"], F32)
    up_w = np.asarray(inputs["up_w"], F32)
    down_w = np.asarray(inputs["down_w"], F32)
    sh_gate_w = np.asarray(inputs["sh_gate_w"], F32)
    sh_up_w = np.asarray(inputs["sh_up_w"], F32)
    sh_down_w = np.asarray(inputs["sh_down_w"], F32)

    in_maps, dispatch = [], []
    for n in range(NEXP):
        idx = np.nonzero((tidx == n).any(axis=1))[0]
        cnt = len(idx)
        assert cnt <= cap
        dispatch.append((idx, cnt))
        hTe = np.zeros((H, cap), F32)
        hTe[:, :cnt] = hT[:, idx]
        hbe_in = np.ascontiguousarray(
            hTe.reshape(KO, P, cap).transpose(1, 0, 2).astype(BF16)
        )
        hfc_in = np.ascontiguousarray(
            hTe.reshape(KO, P, cb_n, P).transpose(2, 1, 0, 3)
        )
        # expert weights: [fb, p(h-inner), ko(h-outer), f'] layouts
        gw4 = gate_w[n].reshape(FBN, P, KO, P)       # (fb, f', ko, p)
        gwl_in = np.ascontiguousarray(gw4.transpose(0, 3, 2, 1).astype(BF16))
        uw4 = up_w[n].reshape(FBN, P, KO, P)
        uwl_in = np.ascontiguousarray(uw4.transpose(0, 3, 2, 1).astype(BF16))
        # shared expert slice, padded 352 -> 384 channels
        shg = np.zeros((FSLP, H), F32)
        shg[:FSL] = sh_gate_w[n * FSL : (n + 1) * FSL]
        sgl_in = np.ascontiguousarray(
            shg.reshape(SBN, P, KO, P).transpose(0, 3, 2, 1).astype(BF16)
        )
        shu = np.zeros((FSLP, H), F32)
        shu[:FSL] = sh_up_w[n * FSL : (n + 1) * FSL]
        sul_in = np.ascontiguousarray(
            shu.reshape(SBN, P, KO, P).transpose(0, 3, 2, 1).astype(BF16)
        )
        # down weights: [hb, p(f-inner), fb, h'] layouts
        dw4 = down_w[n].reshape(HCH, 512, FBN, P)    # (hb, h', fb, f')
        dwl_in = np.ascontiguousarray(dw4.transpose(0, 3, 2, 1).astype(BF16))
        sd = np.zeros((H, FSLP), F32)
        sd[:, :FSL] = sh_down_w[:, n * FSL : (n + 1) * FSL]
        sdl_in = np.ascontiguousarray(
            sd.reshape(HCH, 512, SBN, P).transpose(0, 3, 2, 1).astype(BF16)
        )
        esel_in = np.zeros((P, NEXP), F32)
        esel_in[:, n] = 1.0
        in_maps.append({
            "hbf": hbf_in, "hbe": hbe_in, "hfc": hfc_in, "gw8": gw8_in,
            "esel": esel_in, "gwl": gwl_in, "uwl": uwl_in, "sgl": sgl_in,
            "sul": sul_in, "dwl": dwl_in, "sdl": sdl_in,
        })
    return in_maps, dispatch


def run(inputs, trace=False, **kwargs):
    h = np.asarray(inputs["hidden_states"], F32).reshape(T, H)
    tidx = _host_route(h, np.asarray(inputs["gate_weight"], F32))
    max_load = int(np.bincount(tidx.ravel(), minlength=NEXP).max())
    capc = -(-max_load // 64) * 64          # computed token columns
    cap = -(-capc // P) * P                 # padded layout capacity
    nc = _get_nc(cap, capc)
    in_maps, dispatch = make_in_maps(inputs, cap)
    res = run_bass_kernel_spmd(
        nc, in_maps, core_ids=list(range(NEXP)), trace=trace, **kwargs
    )
    total = res.results[0]["outs"].astype(F32)
    for i in range(1, NEXP):
        total = total + res.results[i]["outs"]
    for n in range(NEXP):
        idx, cnt = dispatch[n]
        total[idx] += res.results[n]["oute"][:cnt]
    return total.reshape(B, S, H), res


def kernel(**inputs):
    out, _ = run(inputs)
    return out


# revision 25
# speedup vs baseline: 2.2296x; 1.0421x over previous
"""DeepSeek-V2-style MoE kernel for 8 Trainium2 NeuronCores.

Sharding strategy (expert-parallel, SPARSE dispatch + shared-expert
channel-parallel):
  - The host replicates the router's top-2 selection (cheap [T,8] matmul
    in fp32 numpy) ONLY to build per-expert compacted token lists; every
    value that reaches the output is computed on-device, including the
    softmax/top-2 combine weights themselves.
  - Core n runs routed expert n on just the tokens routed to it (padded
    to a fixed capacity CAP, a multiple of 128), weighted by that
    expert's per-token combine weight computed on-device from fp32
    logits (group-limited top-2, same as dense baseline).
  - The always-on shared expert's intermediate dim FS=2816 is sharded
    8-ways (352 channels/core, padded to 384); each core's shared
    output over all T tokens is a partial sum.
  - Each core emits oute [CAP, H] (weighted expert rows, compacted
    order) and outs [T, H] (shared partial); the host sums the shared
    partials and scatter-adds the expert rows.

Heavy matmuls run in bf16 (fp32 PSUM accumulation). Weight-stationary
ko-outer loops amortize LDWEIGHTS over the token chunks; routing is
batched so the scalar engine's Exp table is loaded once.
"""

from contextlib import ExitStack

import numpy as np
import ml_dtypes

import concourse.bass as bass
import concourse.tile as tile
from concourse import bacc, mybir
from concourse.bass_utils import run_bass_kernel_spmd

BF16 = ml_dtypes.bfloat16
F32 = np.float32

P = 128
B, S, H, F, FS, NEXP = 2, 1024, 2048, 1408, 2816, 8
T = B * S                      # 2048 tokens
FSL = FS // NEXP               # 352 shared channels per core
FSLP = 384                     # padded to a multiple of 128
KO = H // P                    # 16 contraction chunks over H
TB = T // P                    # 16 token blocks of 128
TCH = T // 512                 # 4 token chunks of 512
HCH = H // 512                 # 4 output chunks of 512
FBN = F // P                   # 11 expert f-blocks
SBN = FSLP // P                # 3 shared fs-blocks
CAP0 = 640                     # default expert token capacity (5 blocks)

_X = mybir.AxisListType.X
_ALU = mybir.AluOpType
_ACTF = mybir.ActivationFunctionType
_F32 = mybir.dt.float32
_F32R = mybir.dt.float32r
_BF16 = mybir.dt.bfloat16

_CACHED_NC = {}


def _chunks(n, w=512):
    """Split n columns into <=w-wide matmul chunks."""
    out, off = [], 0
    while off < n:
        ln = min(w, n - off)
        out.append((off, ln))
        off += ln
    return out


def _build_body(ctx, tc, cap, capc):
    nc = tc.nc
    cb_n = cap // P
    hbf_d = nc.dram_tensor("hbf", [P, KO, T], _BF16, kind="ExternalInput").ap()
    hbe_d = nc.dram_tensor("hbe", [P, KO, cap], _BF16, kind="ExternalInput").ap()
    hfall_d = nc.dram_tensor("hfall", [P, KO, cap], _F32R,
                             kind="ExternalInput").ap()
    gw8_d = nc.dram_tensor("gw8", [P, KO, NEXP], _F32R,
                           kind="ExternalInput").ap()
    ident_d = nc.dram_tensor("ident", [NEXP, NEXP], _F32,
                             kind="ExternalInput").ap()
    esel_d = nc.dram_tensor("esel", [P, NEXP], _F32, kind="ExternalInput").ap()
    gwl_d = nc.dram_tensor("gwl", [FBN, P, KO, P], _BF16, kind="ExternalInput").ap()
    uwl_d = nc.dram_tensor("uwl", [FBN, P, KO, P], _BF16, kind="ExternalInput").ap()
    sgl_d = nc.dram_tensor("sgl", [SBN, P, KO, P], _BF16, kind="ExternalInput").ap()
    sul_d = nc.dram_tensor("sul", [SBN, P, KO, P], _BF16, kind="ExternalInput").ap()
    dwl_d = nc.dram_tensor("dwl", [HCH, P, FBN, 512], _BF16, kind="ExternalInput").ap()
    sdl_d = nc.dram_tensor("sdl", [HCH, P, SBN, 512], _BF16, kind="ExternalInput").ap()
    oute_d = nc.dram_tensor("oute", [cap, H], _F32, kind="ExternalOutput").ap()
    outs_d = nc.dram_tensor("outs", [T, H], _F32, kind="ExternalOutput").ap()

    consts = ctx.enter_context(tc.tile_pool(name="consts", bufs=1))
    a_pool = ctx.enter_context(tc.tile_pool(name="apool", bufs=1))
    wpool = ctx.enter_context(tc.tile_pool(name="wpool", bufs=2))
    rpool = ctx.enter_context(tc.tile_pool(name="rpool", bufs=2))
    sgpool = ctx.enter_context(tc.tile_pool(name="sgpool", bufs=3))
    dpool = ctx.enter_context(tc.tile_pool(name="dpool", bufs=2))
    opool = ctx.enter_context(tc.tile_pool(name="opool", bufs=3))
    mmp = ctx.enter_context(tc.tile_pool(name="mmp", bufs=1, space="PSUM"))

    gw8_sb = consts.tile([P, KO, NEXP], _F32R)
    nc.sync.dma_start(gw8_sb[:], gw8_d[:])
    esel_sb = consts.tile([P, NEXP], _F32)
    nc.sync.dma_start(esel_sb[:], esel_d[:])
    ident_sb = consts.tile([NEXP, NEXP], _F32)
    nc.sync.dma_start(ident_sb[:], ident_d[:])
    w_cap = consts.tile([P, cb_n], _F32)

    hbe_sb = a_pool.tile([P, KO, cap], _BF16)
    nc.sync.dma_start(hbe_sb[:], hbe_d[:])
    aTe = a_pool.tile([P, FBN, capc], _BF16)
    ash = a_pool.tile([P, SBN, T], _BF16)
    logits_sb = a_pool.tile([P, cap], _F32)

    def ffn_unit(wg_src, wu_src, dst, dst_blk, src_sb, chunks):
        """Gate/up matmuls + silu*up for one 128-wide block of the
        intermediate dim. ko-outer: each LDWEIGHTS feeds len(chunks)
        matmuls."""
        # weight DMAs dispatch from the Activation hwdge queue so their
        # pool-slot gating never stalls the bulk stream on the SP queue
        wg_t = wpool.tile([P, KO, P], _BF16, tag="wg", name="wg_t")
        nc.scalar.dma_start(wg_t[:], wg_src)
        wu_t = wpool.tile([P, KO, P], _BF16, tag="wu", name="wu_t")
        nc.scalar.dma_start(wu_t[:], wu_src)
        pgs = [mmp.tile([P, ln], _F32, tag="pg", bufs=4, name=f"pg{i}")
               for i, (_, ln) in enumerate(chunks)]
        for ko in range(KO):
            for i, (off, ln) in enumerate(chunks):
                nc.tensor.matmul(
                    pgs[i][:], wg_t[:, ko, :],
                    src_sb[:, ko, off:off + ln],
                    start=(ko == 0), stop=(ko == KO - 1),
                )
        pus = [mmp.tile([P, ln], _F32, tag="pu", bufs=4, name=f"pu{i}")
               for i, (_, ln) in enumerate(chunks)]
        for ko in range(KO):
            for i, (off, ln) in enumerate(chunks):
                nc.tensor.matmul(
                    pus[i][:], wu_t[:, ko, :],
                    src_sb[:, ko, off:off + ln],
                    start=(ko == 0), stop=(ko == KO - 1),
                )
        for i, (off, ln) in enumerate(chunks):
            sg = sgpool.tile([P, 512], _F32, tag="sg", name="sg")
            nc.scalar.activation(sg[:, :ln], pgs[i][:], _ACTF.Sigmoid)
            nc.vector.tensor_tensor(sg[:, :ln], sg[:, :ln], pgs[i][:],
                                    _ALU.mult)
            nc.vector.tensor_tensor(dst[:, dst_blk, off:off + ln],
                                    sg[:, :ln], pus[i][:], _ALU.mult)

    def routing_chain(j):
        """Router for compacted token block j: transpose the fp32r logits
        back to token-major, then softmax -> group-limited top-2 -> this
        core's combine-weight column w_cap[:, j]."""
        jsl = slice(j * P, (j + 1) * P)
        pl = mmp.tile([P, NEXP], _F32, tag="pu", bufs=4, name="pl")
        nc.tensor.matmul(pl[:], logits_sb[0:NEXP, jsl], ident_sb[:],
                         start=True, stop=True)
        negmx = rpool.tile([P, 1], _F32, tag="negmx", name="negmx")
        nc.vector.tensor_reduce(negmx[:], pl[:], _X, _ALU.max, negate=True)
        ssum = rpool.tile([P, 1], _F32, tag="ssum", name="ssum")
        sc = rpool.tile([P, NEXP], _F32, tag="sc", name="sc")
        nc.scalar.activation(
            sc[:], pl[:], _ACTF.Exp, bias=negmx[:, 0:1], scale=1.0,
            accum_out=ssum[:, 0:1],
        )
        rec = rpool.tile([P, 1], _F32, tag="rec", name="rec")
        nc.vector.reciprocal(rec[:], ssum[:])
        sc2 = rpool.tile([P, NEXP], _F32, tag="sc2", name="sc2")
        nc.vector.tensor_scalar_mul(sc2[:], sc[:], rec[:, 0:1])
        # group scores: max over pairs of adjacent experts -> [P, 4]
        g = rpool.tile([P, 4], _F32, tag="g", name="g")
        nc.vector.tensor_reduce(
            g[:], sc2.rearrange("p (g e) -> p g e", e=2), _X, _ALU.max
        )
        m1g = rpool.tile([P, 1], _F32, tag="m1g", name="m1g")
        nc.vector.tensor_reduce(m1g[:], g[:], _X, _ALU.max)
        is1 = rpool.tile([P, 4], _F32, tag="is1", name="is1")
        nc.vector.tensor_scalar(is1[:], g[:], m1g[:, 0:1], None, _ALU.is_ge)
        gm = rpool.tile([P, 4], _F32, tag="gm", name="gm")
        nc.vector.scalar_tensor_tensor(
            gm[:], is1[:], -1e30, g[:], _ALU.mult, _ALU.add
        )
        m2g = rpool.tile([P, 1], _F32, tag="m2g", name="m2g")
        nc.vector.tensor_reduce(m2g[:], gm[:], _X, _ALU.max)
        gmask = rpool.tile([P, 4], _F32, tag="gmask", name="gmask")
        nc.vector.tensor_scalar(gmask[:], g[:], m2g[:, 0:1], None, _ALU.is_ge)
        smask = rpool.tile([P, NEXP], _F32, tag="smask", name="smask")
        sm_v = smask.rearrange("p (g e) -> p g e", e=2)
        nc.vector.tensor_copy(sm_v[:, :, 0], gmask[:])
        nc.vector.tensor_copy(sm_v[:, :, 1], gmask[:])
        msk = rpool.tile([P, NEXP], _F32, tag="msk", name="msk")
        nc.vector.tensor_tensor(msk[:], sc2[:], smask[:], _ALU.mult)
        m1e = rpool.tile([P, 1], _F32, tag="m1e", name="m1e")
        nc.vector.tensor_reduce(m1e[:], msk[:], _X, _ALU.max)
        is1e = rpool.tile([P, NEXP], _F32, tag="is1e", name="is1e")
        nc.vector.tensor_scalar(is1e[:], msk[:], m1e[:, 0:1], None, _ALU.is_ge)
        me = rpool.tile([P, NEXP], _F32, tag="me", name="me")
        nc.vector.scalar_tensor_tensor(
            me[:], is1e[:], -1e30, msk[:], _ALU.mult, _ALU.add
        )
        m2e = rpool.tile([P, 1], _F32, tag="m2e", name="m2e")
        nc.vector.tensor_reduce(m2e[:], me[:], _X, _ALU.max)
        wsel = rpool.tile([P, NEXP], _F32, tag="wsel", name="wsel")
        nc.vector.tensor_scalar(wsel[:], msk[:], m2e[:, 0:1], None, _ALU.is_ge)
        wall = rpool.tile([P, NEXP], _F32, tag="wall", name="wall")
        nc.vector.tensor_tensor(wall[:], msk[:], wsel[:], _ALU.mult)
        tmp8 = rpool.tile([P, NEXP], _F32, tag="tmp8", name="tmp8")
        nc.vector.tensor_tensor(tmp8[:], wall[:], esel_sb[:], _ALU.mult)
        nc.vector.tensor_reduce(w_cap[:, j : j + 1], tmp8[:], _X, _ALU.add)

    # ---- M-expert head: first two units start as soon as hbe+weights land
    e_chunks = _chunks(capc)
    for fb in range(min(2, FBN)):
        ffn_unit(gwl_d[fb], uwl_d[fb], aTe, fb, hbe_sb, e_chunks)
    # ---- R phase: fp32r logits in [8, tok] layout (full-speed >=256-wide
    # moving), transposed per 128-block via a tiny identity matmul. The
    # fp32r activations live in a scoped right-side pool released before
    # hbf loads, so the two never coexist in SBUF.
    with tc.tile_pool(name="hfp", bufs=1, side="right") as hfp:
        r_chunks = _chunks(cap, 256)
        for c, (off, ln) in enumerate(r_chunks):
            hf_c = hfp.tile([P, KO, ln], _F32R, name=f"hf_c{c}")
            nc.sync.dma_start(hf_c[:], hfall_d[:, :, off:off + ln])
            plg = mmp.tile([NEXP, ln], _F32, tag="pu", bufs=4,
                           name=f"plg{c}")
            for ko in range(KO):
                nc.tensor.matmul(
                    plg[:], gw8_sb[:, ko, :], hf_c[:, ko, :],
                    start=(ko == 0), stop=(ko == KO - 1),
                )
            nc.vector.tensor_copy(logits_sb[0:NEXP, off:off + ln], plg[:])
        for j in range(cb_n):
            routing_chain(j)
    for fb in range(2, FBN):
        ffn_unit(gwl_d[fb], uwl_d[fb], aTe, fb, hbe_sb, e_chunks)
    # hbf is only needed by the shared-expert units; its pool opens after
    # the fp32r pool above is released
    hbf_pool = ctx.enter_context(tc.tile_pool(name="hbfp", bufs=1))
    hbf_sb = hbf_pool.tile([P, KO, T], _BF16)
    nc.sync.dma_start(hbf_sb[:], hbf_d[:])
    s_chunks = _chunks(T)
    for sb in range(SBN):
        ffn_unit(sgl_d[sb], sul_d[sb], ash, sb, hbf_sb, s_chunks)
    # ---- D phase: down-projections, weight, write out ----
    e_blocks = _chunks(capc, P)
    for hb in range(HCH):
        sd_t = dpool.tile([P, SBN, 512], _BF16, tag="sd", name="sd_t")
        nc.sync.dma_start(sd_t[:], sdl_d[hb])
        dw_t = dpool.tile([P, FBN, 512], _BF16, tag="dw", name="dw_t",
                          bufs=3)
        nc.sync.dma_start(dw_t[:], dwl_d[hb])
        # shared first: its long copy+DMA consumer chain drains during the
        # expert blocks, shortening the kernel tail
        for tb in range(TB):
            tbs = slice(tb * P, (tb + 1) * P)
            ps = mmp.tile([P, 512], _F32, tag="pu", bufs=4, name="ps")
            for sb in range(SBN):
                nc.tensor.matmul(
                    ps[:], ash[:, sb, tbs], sd_t[:, sb, :],
                    start=(sb == 0), stop=(sb == SBN - 1),
                )
            os_ = opool.tile([P, 512], _F32, tag="os", name="os_")
            nc.vector.tensor_copy(os_[:], ps[:])
            eng = nc.scalar if tb % 2 else nc.sync
            eng.dma_start(outs_d[tbs, hb * 512:(hb + 1) * 512], os_[:])
        for cb, (off, ln) in enumerate(e_blocks):
            cbs = slice(off, off + ln)
            pe = mmp.tile([P, 512], _F32, tag="pg", bufs=4, name="pe")
            for fb in range(FBN):
                nc.tensor.matmul(
                    pe[:ln, :], aTe[:, fb, cbs], dw_t[:, fb, :],
                    start=(fb == 0), stop=(fb == FBN - 1),
                )
            oe = opool.tile([P, 512], _F32, tag="oe", name="oe", bufs=2)
            nc.vector.tensor_scalar_mul(oe[:ln, :], pe[:ln, :],
                                        w_cap[:ln, cb : cb + 1])
            eng = nc.scalar if cb % 2 else nc.sync
            eng.dma_start(oute_d[cbs, hb * 512:(hb + 1) * 512], oe[:ln, :])


def build_program(cap=CAP0, capc=CAP0):
    nc = bacc.Bacc("TRN2", target_bir_lowering=False, debug=False)
    with tile.TileContext(nc) as tc:
        with ExitStack() as ctx:
            _build_body(ctx, tc, cap, capc)
    nc.compile()
    return nc


def _get_nc(cap=CAP0, capc=CAP0):
    if (cap, capc) not in _CACHED_NC:
        _CACHED_NC[(cap, capc)] = build_program(cap, capc)
    return _CACHED_NC[(cap, capc)]


def _host_route(h, gw):
    """Replicate the reference router's top-2 selection (fp32) to build
    the dispatch. Only token->expert ASSIGNMENT comes from here; the
    combine weights used in the output are computed on-device."""
    logits = (h @ gw.T).astype(F32)                       # [T, 8]
    m = logits.max(-1, keepdims=True)
    e = np.exp(logits - m, dtype=F32)
    sc = e / e.sum(-1, keepdims=True)
    gs = sc.reshape(-1, 4, 2).max(-1)                     # [T, 4]
    gidx = np.argsort(-gs, axis=1, kind="stable")[:, :2]
    gmask = np.zeros_like(gs)
    np.put_along_axis(gmask, gidx, 1.0, axis=1)
    smask = np.repeat(gmask, 2, axis=1)                   # [T, 8]
    masked = np.where(smask > 0, sc, 0.0)
    tidx = np.argsort(-masked, axis=1, kind="stable")[:, :2]
    return tidx


def make_in_maps(inputs, cap):
    """Host-side shard/layout prep: returns per-core input dicts and the
    per-expert (indices, count) used to unshard."""
    h = np.asarray(inputs["hidden_states"], F32).reshape(T, H)
    hT = np.ascontiguousarray(h.T)                              # [H, T]
    hbf_in = np.ascontiguousarray(
        hT.reshape(KO, P, T).transpose(1, 0, 2).astype(BF16)
    )
    gw = np.asarray(inputs["gate_weight"], F32)
    gw8T = gw.T                                                 # [H, 8]
    gw8_in = np.ascontiguousarray(gw8T.reshape(KO, P, NEXP).transpose(1, 0, 2))

    tidx = _host_route(h, gw)
    cb_n = cap // P

    gate_w = np.asarray(inputs["gate_w"], F32)
    up_w = np.asarray(inputs["up_w"], F32)
    down_w = np.asarray(inputs["down_w"], F32)
    sh_gate_w = np.asarray(inputs["sh_gate_w"], F32)
    sh_up_w = np.asarray(inputs["sh_up_w"], F32)
    sh_down_w = np.asarray(inputs["sh_down_w"], F32)

    in_maps, dispatch = [], []
    for n in range(NEXP):
        idx = np.nonzero((tidx == n).any(axis=1))[0]
        cnt = len(idx)
        assert cnt <= cap
        dispatch.append((idx, cnt))
        hTe = np.zeros((H, cap), F32)
        hTe[:, :cnt] = hT[:, idx]
        hbe_in = np.ascontiguousarray(
            hTe.reshape(KO, P, cap).transpose(1, 0, 2).astype(BF16)
        )
        hfall_in = np.ascontiguousarray(
            hTe.reshape(KO, P, cap).transpose(1, 0, 2)
        )
        # expert weights: [fb, p(h-inner), ko(h-outer), f'] layouts
        gw4 = gate_w[n].reshape(FBN, P, KO, P)       # (fb, f', ko, p)
        gwl_in = np.ascontiguousarray(gw4.transpose(0, 3, 2, 1).astype(BF16))
        uw4 = up_w[n].reshape(FBN, P, KO, P)
        uwl_in = np.ascontiguousarray(uw4.transpose(0, 3, 2, 1).astype(BF16))
        # shared expert slice, padded 352 -> 384 channels
        shg = np.zeros((FSLP, H), F32)
        shg[:FSL] = sh_gate_w[n * FSL : (n + 1) * FSL]
        sgl_in = np.ascontiguousarray(
            shg.reshape(SBN, P, KO, P).transpose(0, 3, 2, 1).astype(BF16)
        )
        shu = np.zeros((FSLP, H), F32)
        shu[:FSL] = sh_up_w[n * FSL : (n + 1) * FSL]
        sul_in = np.ascontiguousarray(
            shu.reshape(SBN, P, KO, P).transpose(0, 3, 2, 1).astype(BF16)
        )
        # down weights: [hb, p(f-inner), fb, h'] layouts
        dw4 = down_w[n].reshape(HCH, 512, FBN, P)    # (hb, h', fb, f')
        dwl_in = np.ascontiguousarray(dw4.transpose(0, 3, 2, 1).astype(BF16))
        sd = np.zeros((H, FSLP), F32)
        sd[:, :FSL] = sh_down_w[:, n * FSL : (n + 1) * FSL]
        sdl_in = np.ascontiguousarray(
            sd.reshape(HCH, 512, SBN, P).transpose(0, 3, 2, 1).astype(BF16)
        )
        esel_in = np.zeros((P, NEXP), F32)
        esel_in[:, n] = 1.0
        in_maps.append({
            "hbf": hbf_in, "hbe": hbe_in, "hfall": hfall_in, "gw8": gw8_in,
            "ident": np.eye(NEXP, dtype=F32), "esel": esel_in,
            "gwl": gwl_in, "uwl": uwl_in, "sgl": sgl_in,
            "sul": sul_in, "dwl": dwl_in, "sdl": sdl_in,
        })
    return in_maps, dispatch


def run(inputs, trace=False, **kwargs):
    h = np.asarray(inputs["hidden_states"], F32).reshape(T, H)
    tidx = _host_route(h, np.asarray(inputs["gate_weight"], F32))
    max_load = int(np.bincount(tidx.ravel(), minlength=NEXP).max())
    capc = -(-max_load // 64) * 64          # computed token columns
    cap = -(-capc // P) * P                 # padded layout capacity
    nc = _get_nc(cap, capc)
    in_maps, dispatch = make_in_maps(inputs, cap)
    res = run_bass_kernel_spmd(
        nc, in_maps, core_ids=list(range(NEXP)), trace=trace, **kwargs
    )
    total = res.results[0]["outs"].astype(F32)
    for i in range(1, NEXP):
        total = total + res.results[i]["outs"]
    for n in range(NEXP):
        idx, cnt = dispatch[n]
        total[idx] += res.results[n]["oute"][:cnt]
    return total.reshape(B, S, H), res


def kernel(**inputs):
    out, _ = run(inputs)
    return out
